# revision 6
# baseline (speedup 1.0000x reference)
"""Trainium2 Bass kernel for nn_Corr (correlation-attention module).

Math (per batch n):
    f1 = 0.5*(w1 @ feat + b1)        # [4, 6400]   feat = feature_in[n] flattened
    f2 =      w2 @ feat + b2         # [4, 6400]
    S  = f1^T @ f2                   # [6400, 6400]  (0.5 = 1/sqrt(nclass) folded into f1)
    A  = softmax(S, axis=1)          # row softmax (over q)
    V  = bilinear_resize(out[n])     # [4, 6400]
    fina[c, q] = sum_p V[c, p]/Z_p * exp(S[p, q])

Sharding: 2 batches x 4 p-shards (rows of S) = 8 cores. Each core produces a
partial fina over its 1600 p-rows; host sums the 4 partials per batch.

Device kernel per core, ScalarE(exp)-bound design (~6.6us per 128-row p-block):
  - S chunk = matmul(lhsT=f1pad[:, block], rhs=f2pad[:, qchunk]) with M=128,
    K=128 (12 live rows: fp16 hi/lo split), N=512 per MM (one PSUM bank).
  - Two ping-pong PSUM regions A=[128,2048] (4 banks) / B=[128,1536] (3 banks)
    give 4 wide EXP activations per block (2048/1536/2048/768), each with
    accum_out producing the row-sum partials Z for free.
  - fina accumulates IN PSUM (1 bank, [128,512]) across all 13 p-blocks:
    MM s (q-subgroup of 512) uses a zero-padded [128,128] lhsT slice of a
    sliding window buffer holding V/Z at cols 48:52, so output partitions
    4s..4s+4 catch class c of q-subgroup s and all other partitions get +0.
  - Emission order per iteration: S(pb) MMs+EXP, then fina(pb-1) MMs, then
    Z/vt scaling on VectorE -> PE never blocks the ACT stream.
"""

import numpy as np

N_CORES = 8
NB = 2          # batches
NCLS = 4        # nclass
C_IN = 32
H = W = 80
HW = H * W      # 6400
NSH = 4         # p-shards per batch
PSH = HW // NSH  # 1600 p rows per shard
PBLK = 13        # p blocks of 128 (1664 = 13*128, last 64 rows are zero-pad)
PPAD = PBLK * 128  # 1664
NSUB = 13        # fina q-subgroups of 512 (12*512 + 256)
# S chunk widths per block: ping-pong regions A (4 banks) / B (3 banks)
CHUNKS = ((0, 2048, "A"), (2048, 1536, "B"), (3584, 2048, "A"), (5632, 768, "B"))

_CACHE = {}


def _resize_bilinear_ac(x, h_out, w_out):
    """numpy mirror of the reference's align_corners=True bilinear resize."""
    n, c, h, w = x.shape
    if (h, w) == (h_out, w_out):
        return x
    ys = np.linspace(0.0, h - 1.0, h_out, dtype=np.float32)
    xs = np.linspace(0.0, w - 1.0, w_out, dtype=np.float32)
    y0 = np.floor(ys).astype(np.int32)
    x0 = np.floor(xs).astype(np.int32)
    y1 = np.minimum(y0 + 1, h - 1)
    x1 = np.minimum(x0 + 1, w - 1)
    wy = (ys - y0.astype(np.float32))[None, None, :, None]
    wx = (xs - x0.astype(np.float32))[None, None, None, :]
    g = lambda yi, xi: x[:, :, yi, :][:, :, :, xi]
    top = g(y0, x0) * (1.0 - wx) + g(y0, x1) * wx
    bot = g(y1, x0) * (1.0 - wx) + g(y1, x1) * wx
    return (top * (1.0 - wy) + bot * wy).astype(np.float32)


def _build_bass():
    import concourse.bass as bass
    import concourse.tile as tile
    from concourse import bacc, mybir

    f32 = mybir.dt.float32
    f16 = mybir.dt.float16

    nc = bacc.Bacc(
        "TRN2", target_bir_lowering=False, debug=False, num_devices=N_CORES
    )

    f1p_d = nc.dram_tensor("f1p", [12, PPAD], f16, kind="ExternalInput")
    f2p_d = nc.dram_tensor("f2p", [12, HW], f16, kind="ExternalInput")
    vt_d = nc.dram_tensor("vt", [128, NCLS * PBLK], f32, kind="ExternalInput")
    res_d = nc.dram_tensor("res", [4 * NSUB, 512], f32, kind="ExternalOutput")

    EXP = mybir.ActivationFunctionType.Exp
    ADD = mybir.AluOpType.add
    MULT = mybir.AluOpType.mult
    AXX = mybir.AxisListType.X

    with tile.TileContext(nc) as tc:
        with (
            tc.tile_pool(name="const", bufs=1) as cpool,
            tc.tile_pool(name="estrip", bufs=2) as epool,
            tc.tile_pool(name="zpool", bufs=2) as zpool,
            tc.tile_pool(name="spsum", bufs=1, space="PSUM") as spool,
            tc.tile_pool(name="fpsum", bufs=1, space="PSUM") as fpool,
        ):
            # K=12 contraction: exact-size APs, no zero-padding of K rows
            f1s = cpool.tile([12, PPAD], f16, tag="f1s")
            f2s = cpool.tile([12, HW], f16, tag="f2s")
            vts = cpool.tile([128, NCLS * PBLK], f32, tag="vts")
            # sliding-window fina weights: V/Z at cols 48:52, zeros elsewhere;
            # MM s uses the [128,64] slice starting at col 48-4s
            vtbA = cpool.tile([128, 112], f16, tag="vtbA")
            vtbB = cpool.tile([128, 112], f16, tag="vtbB")
            bneg = cpool.tile([128, 1], f32, tag="bneg")
            fout = cpool.tile([128, 512], f32, tag="fout")
            dumm = cpool.tile([128, 1], f32, tag="dumm")

            # persistent fina accumulator: 1 PSUM bank
            fps = fpool.tile([128, 512], f32, tag="fps")

            nc.sync.dma_start(out=f1s[:, :], in_=f1p_d[:, :])
            nc.sync.dma_start(out=f2s[:, :], in_=f2p_d[:, :])
            nc.sync.dma_start(out=vts[:, :], in_=vt_d[:, :])
            nc.gpsimd.memset(bneg[:, :], -5.0)
            # dummy activation: pulls the ~2.7us exp table load under the DMAs
            nc.scalar.activation(dumm[:, 0:1], bneg[:, 0:1], EXP)
            nc.gpsimd.memset(vtbA[:, :], 0.0)
            nc.gpsimd.memset(vtbB[:, :], 0.0)

            def emit_fina(pb):
                vtb = vtbA if pb % 2 == 0 else vtbB
                et = et_of[pb]
                for s in range(NSUB):
                    w = min(512, HW - 512 * s)
                    nc.tensor.matmul(
                        fps[0:64, 0:w],
                        lhsT=vtb[:, 48 - 4 * s : 112 - 4 * s],
                        rhs=et[:, 512 * s : 512 * s + w],
                        start=(pb == 0 and s == 0),
                        stop=(pb == PBLK - 1 and s == NSUB - 1),
                        skip_group_check=True,
                    )

            et_of = {}
            for pb in range(PBLK):
                et = epool.tile([128, HW], f16, tag="et")
                et_of[pb] = et
                zparts = zpool.tile([128, 4], f32, tag="zparts")
                rz = zpool.tile([128, 1], f32, tag="rz")

                for ci, (q0, width, reg) in enumerate(CHUNKS):
                    # fina(pb-1) emitted before the last S chunk: frees the
                    # PE to start S(pb+1) right after ACT(pb) region-A read
                    if ci == 3 and pb > 0:
                        emit_fina(pb - 1)
                    st = spool.tile([128, 2048 if reg == "A" else 1536], f32,
                                    tag="st" + reg)
                    for off in range(0, width, 512):
                        w = min(512, width - off)
                        # M=64 halves: 16-bit moving operand streams 2
                        # cols/cycle only when M <= 64 (PSUM drain port)
                        for h in range(2):
                            nc.tensor.matmul(
                                st[64 * h : 64 * h + 64, off : off + w],
                                lhsT=f1s[:, 128 * pb + 64 * h : 128 * pb + 64 * h + 64],
                                rhs=f2s[:, q0 + off : q0 + off + w],
                                start=True,
                                stop=True,
                                skip_group_check=True,
                            )
                    # bias -5: keeps exp within fp16 range (softmax is
                    # shift-invariant; Z accumulates the same shifted values)
                    nc.scalar.activation(
                        et[:, q0 : q0 + width],
                        st[:, 0:width],
                        EXP,
                        bias=bneg[:, 0:1],
                        accum_out=zparts[:, ci : ci + 1],
                    )

                # Z = sum of chunk partials; vtb cols 48:52 = V[:, block]/Z
                nc.vector.tensor_reduce(rz[:, 0:1], zparts[:, :], AXX, ADD)
                nc.vector.reciprocal(rz[:, 0:1], rz[:, 0:1])
                vtb = vtbA if pb % 2 == 0 else vtbB
                nc.vector.tensor_scalar(
                    vtb[:, 48:52],
                    vts[:, NCLS * pb : NCLS * pb + NCLS],
                    rz[:, 0:1],
                    2048.0,
                    MULT,
                    MULT,
                )

            emit_fina(PBLK - 1)
            nc.vector.tensor_copy(fout[:, :], fps[:, :])
            nc.sync.dma_start(out=res_d[:, :], in_=fout[0 : 4 * NSUB, :])

    nc.compile()
    return nc


def _get_nc():
    if "nc" not in _CACHE:
        _CACHE["nc"] = _build_bass()
    return _CACHE["nc"]


def _hilo16(x):
    """fp16 high/low split: x ~= hi + lo exactly to ~2^-22 relative."""
    x = np.asarray(x, np.float32)
    hi = x.astype(np.float16)
    lo = (x - hi.astype(np.float32)).astype(np.float16)
    return hi, lo


def _prep_inputs(feature_in, out, w1, b1, w2, b2):
    feature_in = np.asarray(feature_in, np.float32)
    out = np.asarray(out, np.float32)
    w1 = np.asarray(w1, np.float32)
    b1 = np.asarray(b1, np.float32)
    w2 = np.asarray(w2, np.float32)
    b2 = np.asarray(b2, np.float32)

    scale = np.float32(1.0 / np.sqrt(NCLS))
    feat = feature_in.reshape(NB, C_IN, HW)
    # f1 carries the softmax scale; f2 is plain
    f1 = (np.einsum("oc,ncp->nop", w1, feat, dtype=np.float32) + b1[None, :, None]) * scale
    f2 = np.einsum("oc,ncp->nop", w2, feat, dtype=np.float32) + b2[None, :, None]
    f1 = f1.astype(np.float32)
    f2 = f2.astype(np.float32)
    v = _resize_bilinear_ac(out, H, W).reshape(NB, NCLS, HW)

    in_maps = []
    for core in range(N_CORES):
        b, s = divmod(core, NSH)
        p0 = PSH * s
        f1p = np.zeros((12, PPAD), np.float16)
        h1, l1 = _hilo16(f1[b][:, p0 : p0 + PSH])
        f1p[0:4, :PSH] = h1
        f1p[4:8, :PSH] = l1
        f1p[8:12, :PSH] = h1
        h2, l2 = _hilo16(f2[b])
        f2p = np.concatenate([h2, h2, l2], axis=0)  # [12, HW] fp16
        vtp = np.zeros((NCLS, PPAD), np.float32)
        vtp[:, :PSH] = v[b][:, p0 : p0 + PSH]
        # vt[part, 4*pb + c] = V[c, p0 + 128*pb + part]
        vt = vtp.reshape(NCLS, PBLK, 128).transpose(2, 1, 0).reshape(128, PBLK * NCLS)
        in_maps.append(
            {
                "f1p": f1p,
                "f2p": np.ascontiguousarray(f2p),
                "vt": np.ascontiguousarray(vt),
            }
        )
    return in_maps


def _unpack(results):
    """results: list of 8 dicts with 'res' [52, 512] -> fina [2,4,80,80]."""
    fina = np.zeros((NB, NCLS, HW), np.float32)
    for core in range(N_CORES):
        b, _s = divmod(core, NSH)
        res = np.asarray(results[core]["res"], np.float32)  # [52, 512]
        for s in range(NSUB):
            w = min(512, HW - 512 * s)
            fina[b, :, 512 * s : 512 * s + w] += res[4 * s : 4 * s + 4, :w]
    fina *= np.float32(1.0 / 2048.0)
    return fina.reshape(NB, NCLS, H, W)


def run(inputs, trace=False):
    from concourse.bass_utils import run_bass_kernel_spmd

    nc = _get_nc()
    in_maps = _prep_inputs(**inputs)
    r = run_bass_kernel_spmd(nc, in_maps, list(range(N_CORES)), trace=trace)
    return _unpack(r.results), r.exec_time_ns


def kernel(feature_in, out, w1, b1, w2, b2):
    result, _ = run(
        dict(feature_in=feature_in, out=out, w1=w1, b1=b1, w2=w2, b2=b2)
    )
    return result


# revision 12
# speedup vs baseline: 1.1934x; 1.1934x over previous
"""Trainium2 Bass kernel for nn_Corr (correlation-attention module).

Math (per batch n):
    f1 = 0.5*(w1 @ feat + b1)        # [4, 6400]   feat = feature_in[n] flattened
    f2 =      w2 @ feat + b2         # [4, 6400]
    S  = f1^T @ f2                   # [6400, 6400]  (0.5 = 1/sqrt(nclass) folded into f1)
    A  = softmax(S, axis=1)          # row softmax (over q)
    V  = bilinear_resize(out[n])     # [4, 6400]
    fina[c, q] = sum_p V[c, p]/Z_p * exp(S[p, q])

Sharding: 2 batches x 4 p-shards (rows of S) = 8 cores. Each core produces a
partial fina over its 1600 p-rows; host sums the 4 partials per batch.

Device kernel per core, ScalarE(exp)-bound design (~6.6us per 128-row p-block):
  - S chunk = matmul(lhsT=f1pad[:, block], rhs=f2pad[:, qchunk]) with M=128,
    K=128 (12 live rows: fp16 hi/lo split), N=512 per MM (one PSUM bank).
  - Two ping-pong PSUM regions A=[128,2048] (4 banks) / B=[128,1536] (3 banks)
    give 4 wide EXP activations per block (2048/1536/2048/768), each with
    accum_out producing the row-sum partials Z for free.
  - fina accumulates IN PSUM (1 bank, [128,512]) across all 13 p-blocks:
    MM s (q-subgroup of 512) uses a zero-padded [128,128] lhsT slice of a
    sliding window buffer holding V/Z at cols 48:52, so output partitions
    4s..4s+4 catch class c of q-subgroup s and all other partitions get +0.
  - Emission order per iteration: S(pb) MMs+EXP, then fina(pb-1) MMs, then
    Z/vt scaling on VectorE -> PE never blocks the ACT stream.
"""

import numpy as np

N_CORES = 8
NB = 2          # batches
NCLS = 4        # nclass
C_IN = 32
H = W = 80
HW = H * W      # 6400
NSH = 4         # p-shards per batch
PSH = HW // NSH  # 1600 p rows per shard
PBLK = 13        # p blocks of 128 (1664 = 13*128, last 64 rows are zero-pad)
PPAD = PBLK * 128  # 1664
NSUB = 13        # fina q-subgroups of 512 (12*512 + 256)
# S chunk widths per block: ping-pong regions A (4 banks) / B (3 banks)
CHUNKS = ((0, 2048, "A"), (2048, 1536, "B"), (3584, 2048, "A"), (5632, 768, "B"))

_CACHE = {}


def _resize_bilinear_ac(x, h_out, w_out):
    """numpy mirror of the reference's align_corners=True bilinear resize."""
    n, c, h, w = x.shape
    if (h, w) == (h_out, w_out):
        return x
    ys = np.linspace(0.0, h - 1.0, h_out, dtype=np.float32)
    xs = np.linspace(0.0, w - 1.0, w_out, dtype=np.float32)
    y0 = np.floor(ys).astype(np.int32)
    x0 = np.floor(xs).astype(np.int32)
    y1 = np.minimum(y0 + 1, h - 1)
    x1 = np.minimum(x0 + 1, w - 1)
    wy = (ys - y0.astype(np.float32))[None, None, :, None]
    wx = (xs - x0.astype(np.float32))[None, None, None, :]
    g = lambda yi, xi: x[:, :, yi, :][:, :, :, xi]
    top = g(y0, x0) * (1.0 - wx) + g(y0, x1) * wx
    bot = g(y1, x0) * (1.0 - wx) + g(y1, x1) * wx
    return (top * (1.0 - wy) + bot * wy).astype(np.float32)


def _build_bass():
    import concourse.bass as bass
    import concourse.tile as tile
    from concourse import bacc, mybir

    f32 = mybir.dt.float32
    f16 = mybir.dt.float16

    nc = bacc.Bacc(
        "TRN2", target_bir_lowering=False, debug=False, num_devices=N_CORES
    )

    f1p_d = nc.dram_tensor("f1p", [32, PPAD], f16, kind="ExternalInput")
    f2p_d = nc.dram_tensor("f2p", [32, HW], f16, kind="ExternalInput")
    vt_d = nc.dram_tensor("vt", [128, NCLS * PBLK], f32, kind="ExternalInput")
    res_d = nc.dram_tensor("res", [4 * NSUB, 512], f32, kind="ExternalOutput")

    EXP = mybir.ActivationFunctionType.Exp
    ADD = mybir.AluOpType.add
    MULT = mybir.AluOpType.mult
    AXX = mybir.AxisListType.X

    with tile.TileContext(nc) as tc:
        with (
            tc.tile_pool(name="const", bufs=1) as cpool,
            tc.tile_pool(name="estrip", bufs=2) as epool,
            tc.tile_pool(name="zpool", bufs=2) as zpool,
            tc.tile_pool(name="spsum", bufs=1, space="PSUM") as spool,
            tc.tile_pool(name="fpsum", bufs=1, space="PSUM") as fpool,
        ):
            f1s = cpool.tile([128, PPAD], f16, tag="f1s")
            f2s = cpool.tile([128, HW], f16, tag="f2s")
            vts = cpool.tile([128, NCLS * PBLK], f32, tag="vts")
            # sliding-window fina weights: V/Z at cols 48:52, zeros elsewhere;
            # MM s uses the [128,64] slice starting at col 48-4s
            vtbA = cpool.tile([128, 112], f16, tag="vtbA")
            vtbB = cpool.tile([128, 112], f16, tag="vtbB")
            bneg = cpool.tile([128, 1], f32, tag="bneg")
            fout = cpool.tile([128, 512], f32, tag="fout")
            dumm = cpool.tile([128, 1], f32, tag="dumm")

            # persistent fina accumulator: 1 PSUM bank
            fps = fpool.tile([128, 512], f32, tag="fps")

            nc.sync.dma_start(out=f1s[0:32, :], in_=f1p_d[:, :])
            nc.sync.dma_start(out=f2s[0:32, :], in_=f2p_d[:, :])
            nc.sync.dma_start(out=vts[:, :], in_=vt_d[:, :])
            nc.gpsimd.memset(bneg[:, :], -5.0)
            # dummy activation: pulls the ~2.7us exp table load under the DMAs
            nc.scalar.activation(dumm[:, 0:1], bneg[:, 0:1], EXP)
            # zero the K pad rows: host DMA covers 0:32 (rows 12:32 zero),
            # DVE and GpSimd zero the 32-aligned tail in parallel with the
            # DMAs (non-zero partition base caps each op at 32 partitions)
            nc.vector.memset(f2s[32:64, :], 0.0)
            nc.vector.memset(f2s[64:96, :], 0.0)
            nc.vector.memset(f2s[96:128, :], 0.0)
            nc.gpsimd.memset(f1s[32:64, :], 0.0)
            nc.gpsimd.memset(f1s[64:96, :], 0.0)
            nc.gpsimd.memset(f1s[96:128, :], 0.0)
            nc.gpsimd.memset(vtbA[:, :], 0.0)
            nc.gpsimd.memset(vtbB[:, :], 0.0)

            def emit_fina(pb):
                vtb = vtbA if pb % 2 == 0 else vtbB
                et = et_of[pb]
                for s in range(NSUB):
                    w = min(512, HW - 512 * s)
                    nc.tensor.matmul(
                        fps[0:64, 0:w],
                        lhsT=vtb[:, 48 - 4 * s : 112 - 4 * s],
                        rhs=et[:, 512 * s : 512 * s + w],
                        start=(pb == 0 and s == 0),
                        stop=(pb == PBLK - 1 and s == NSUB - 1),
                        skip_group_check=True,
                    )

            et_of = {}
            for pb in range(PBLK):
                et = epool.tile([128, HW], f16, tag="et")
                et_of[pb] = et
                zparts = zpool.tile([128, 4], f32, tag="zparts")
                rz = zpool.tile([128, 1], f32, tag="rz")

                for ci, (q0, width, reg) in enumerate(CHUNKS):
                    # fina(pb-1) emitted before the last S chunk: frees the
                    # PE to start S(pb+1) right after ACT(pb) region-A read
                    if ci == 3 and pb > 0:
                        emit_fina(pb - 1)
                    st = spool.tile([128, 2048 if reg == "A" else 1536], f32,
                                    tag="st" + reg)
                    for off in range(0, width, 512):
                        w = min(512, width - off)
                        # M=64 halves: 16-bit moving operand streams 2
                        # cols/cycle only when M <= 64 (PSUM drain port)
                        for h in range(2):
                            nc.tensor.matmul(
                                st[64 * h : 64 * h + 64, off : off + w],
                                lhsT=f1s[:, 128 * pb + 64 * h : 128 * pb + 64 * h + 64],
                                rhs=f2s[:, q0 + off : q0 + off + w],
                                start=True,
                                stop=True,
                                skip_group_check=True,
                            )
                    # bias -5: keeps exp within fp16 range (softmax is
                    # shift-invariant; Z accumulates the same shifted values)
                    nc.scalar.activation(
                        et[:, q0 : q0 + width],
                        st[:, 0:width],
                        EXP,
                        bias=bneg[:, 0:1],
                        accum_out=zparts[:, ci : ci + 1],
                    )

                # Z = sum of chunk partials; vtb cols 48:52 = V[:, block]/Z
                nc.vector.tensor_reduce(rz[:, 0:1], zparts[:, :], AXX, ADD)
                nc.vector.reciprocal(rz[:, 0:1], rz[:, 0:1])
                vtb = vtbA if pb % 2 == 0 else vtbB
                nc.vector.tensor_scalar(
                    vtb[:, 48:52],
                    vts[:, NCLS * pb : NCLS * pb + NCLS],
                    rz[:, 0:1],
                    2048.0,
                    MULT,
                    MULT,
                )

            emit_fina(PBLK - 1)
            nc.vector.tensor_copy(fout[:, :], fps[:, :])
            nc.sync.dma_start(out=res_d[:, :], in_=fout[0 : 4 * NSUB, :])

    nc.compile()
    return nc


def _get_nc():
    if "nc" not in _CACHE:
        _CACHE["nc"] = _build_bass()
    return _CACHE["nc"]


def _hilo16(x):
    """fp16 high/low split: x ~= hi + lo exactly to ~2^-22 relative."""
    x = np.asarray(x, np.float32)
    hi = x.astype(np.float16)
    lo = (x - hi.astype(np.float32)).astype(np.float16)
    return hi, lo


def _prep_inputs(feature_in, out, w1, b1, w2, b2):
    feature_in = np.asarray(feature_in, np.float32)
    out = np.asarray(out, np.float32)
    w1 = np.asarray(w1, np.float32)
    b1 = np.asarray(b1, np.float32)
    w2 = np.asarray(w2, np.float32)
    b2 = np.asarray(b2, np.float32)

    scale = np.float32(1.0 / np.sqrt(NCLS))
    feat = feature_in.reshape(NB, C_IN, HW)
    # f1 carries the softmax scale; f2 is plain
    f1 = (np.einsum("oc,ncp->nop", w1, feat, dtype=np.float32) + b1[None, :, None]) * scale
    f2 = np.einsum("oc,ncp->nop", w2, feat, dtype=np.float32) + b2[None, :, None]
    f1 = f1.astype(np.float32)
    f2 = f2.astype(np.float32)
    v = _resize_bilinear_ac(out, H, W).reshape(NB, NCLS, HW)

    in_maps = []
    for core in range(N_CORES):
        b, s = divmod(core, NSH)
        p0 = PSH * s
        f1p = np.zeros((32, PPAD), np.float16)
        h1, l1 = _hilo16(f1[b][:, p0 : p0 + PSH])
        f1p[0:4, :PSH] = h1
        f1p[4:8, :PSH] = l1
        f1p[8:12, :PSH] = h1
        h2, l2 = _hilo16(f2[b])
        f2p = np.zeros((32, HW), np.float16)
        f2p[0:12] = np.concatenate([h2, h2, l2], axis=0)
        vtp = np.zeros((NCLS, PPAD), np.float32)
        vtp[:, :PSH] = v[b][:, p0 : p0 + PSH]
        # vt[part, 4*pb + c] = V[c, p0 + 128*pb + part]
        vt = vtp.reshape(NCLS, PBLK, 128).transpose(2, 1, 0).reshape(128, PBLK * NCLS)
        in_maps.append(
            {
                "f1p": f1p,
                "f2p": np.ascontiguousarray(f2p),
                "vt": np.ascontiguousarray(vt),
            }
        )
    return in_maps


def _unpack(results):
    """results: list of 8 dicts with 'res' [52, 512] -> fina [2,4,80,80]."""
    fina = np.zeros((NB, NCLS, HW), np.float32)
    for core in range(N_CORES):
        b, _s = divmod(core, NSH)
        res = np.asarray(results[core]["res"], np.float32)  # [52, 512]
        for s in range(NSUB):
            w = min(512, HW - 512 * s)
            fina[b, :, 512 * s : 512 * s + w] += res[4 * s : 4 * s + 4, :w]
    fina *= np.float32(1.0 / 2048.0)
    return fina.reshape(NB, NCLS, H, W)


def run(inputs, trace=False):
    from concourse.bass_utils import run_bass_kernel_spmd

    nc = _get_nc()
    in_maps = _prep_inputs(**inputs)
    r = run_bass_kernel_spmd(nc, in_maps, list(range(N_CORES)), trace=trace)
    return _unpack(r.results), r.exec_time_ns


def kernel(feature_in, out, w1, b1, w2, b2):
    result, _ = run(
        dict(feature_in=feature_in, out=out, w1=w1, b1=b1, w2=w2, b2=b2)
    )
    return result


# revision 16
# speedup vs baseline: 1.2744x; 1.0679x over previous
"""Trainium2 Bass kernel for nn_Corr (correlation-attention module).

Math (per batch n):
    f1 = 0.5*(w1 @ feat + b1)        # [4, 6400]   feat = feature_in[n] flattened
    f2 =      w2 @ feat + b2         # [4, 6400]
    S  = f1^T @ f2                   # [6400, 6400]  (0.5 = 1/sqrt(nclass) folded into f1)
    A  = softmax(S, axis=1)          # row softmax (over q)
    V  = bilinear_resize(out[n])     # [4, 6400]
    fina[c, q] = sum_p V[c, p]/Z_p * exp(S[p, q])

Sharding: 2 batches x 4 p-shards (rows of S) = 8 cores. Each core produces a
partial fina over its 1600 p-rows; host sums the 4 partials per batch.

Device kernel per core, ScalarE(exp)-bound design (~6.6us per 128-row p-block):
  - S chunk = matmul(lhsT=f1pad[:, block], rhs=f2pad[:, qchunk]) with M=128,
    K=128 (12 live rows: fp16 hi/lo split), N=512 per MM (one PSUM bank).
  - Two ping-pong PSUM regions A=[128,2048] (4 banks) / B=[128,1536] (3 banks)
    give 4 wide EXP activations per block (2048/1536/2048/768), each with
    accum_out producing the row-sum partials Z for free.
  - fina accumulates IN PSUM (1 bank, [128,512]) across all 13 p-blocks:
    MM s (q-subgroup of 512) uses a zero-padded [128,128] lhsT slice of a
    sliding window buffer holding V/Z at cols 48:52, so output partitions
    4s..4s+4 catch class c of q-subgroup s and all other partitions get +0.
  - Emission order per iteration: S(pb) MMs+EXP, then fina(pb-1) MMs, then
    Z/vt scaling on VectorE -> PE never blocks the ACT stream.
"""

import numpy as np

N_CORES = 8
NB = 2          # batches
NCLS = 4        # nclass
C_IN = 32
H = W = 80
HW = H * W      # 6400
NSH = 4         # p-shards per batch
PSH = HW // NSH  # 1600 p rows per shard
PBLK = 13        # p blocks of 128 (1664 = 13*128, last 64 rows are zero-pad)
PPAD = PBLK * 128  # 1664
NSUB = 13        # fina q-subgroups of 512 (12*512 + 256)
# S chunk widths per block: ping-pong regions A (4 banks) / B (3 banks)
CHUNKS = ((0, 2048, "A"), (2048, 1536, "B"), (3584, 2048, "A"), (5632, 768, "B"))

_CACHE = {}


def _resize_bilinear_ac(x, h_out, w_out):
    """numpy mirror of the reference's align_corners=True bilinear resize."""
    n, c, h, w = x.shape
    if (h, w) == (h_out, w_out):
        return x
    ys = np.linspace(0.0, h - 1.0, h_out, dtype=np.float32)
    xs = np.linspace(0.0, w - 1.0, w_out, dtype=np.float32)
    y0 = np.floor(ys).astype(np.int32)
    x0 = np.floor(xs).astype(np.int32)
    y1 = np.minimum(y0 + 1, h - 1)
    x1 = np.minimum(x0 + 1, w - 1)
    wy = (ys - y0.astype(np.float32))[None, None, :, None]
    wx = (xs - x0.astype(np.float32))[None, None, None, :]
    g = lambda yi, xi: x[:, :, yi, :][:, :, :, xi]
    top = g(y0, x0) * (1.0 - wx) + g(y0, x1) * wx
    bot = g(y1, x0) * (1.0 - wx) + g(y1, x1) * wx
    return (top * (1.0 - wy) + bot * wy).astype(np.float32)


def _build_bass():
    import concourse.bass as bass
    import concourse.tile as tile
    from concourse import bacc, mybir

    f32 = mybir.dt.float32
    f16 = mybir.dt.float16

    nc = bacc.Bacc(
        "TRN2", target_bir_lowering=False, debug=False, num_devices=N_CORES
    )

    f1p_d = nc.dram_tensor("f1p", [128, PPAD], f16, kind="ExternalInput")
    f2p_d = nc.dram_tensor("f2p", [128, HW], f16, kind="ExternalInput")
    vt_d = nc.dram_tensor("vt", [128, NCLS * PBLK], f32, kind="ExternalInput")
    res_d = nc.dram_tensor("res", [4 * NSUB, 512], f32, kind="ExternalOutput")

    EXP = mybir.ActivationFunctionType.Exp
    ADD = mybir.AluOpType.add
    MULT = mybir.AluOpType.mult
    AXX = mybir.AxisListType.X

    with tile.TileContext(nc) as tc:
        with (
            tc.tile_pool(name="const", bufs=1) as cpool,
            tc.tile_pool(name="estrip", bufs=2) as epool,
            tc.tile_pool(name="zpool", bufs=2) as zpool,
            tc.tile_pool(name="spsum", bufs=1, space="PSUM") as spool,
            tc.tile_pool(name="fpsum", bufs=1, space="PSUM") as fpool,
        ):
            f1s = cpool.tile([128, PPAD], f16, tag="f1s")
            f2s = cpool.tile([128, HW], f16, tag="f2s")
            vts = cpool.tile([128, NCLS * PBLK], f32, tag="vts")
            # sliding-window fina weights: V/Z at cols 48:52, zeros elsewhere;
            # MM s uses the [128,64] slice starting at col 48-4s
            vtbA = cpool.tile([128, 112], f16, tag="vtbA")
            vtbB = cpool.tile([128, 112], f16, tag="vtbB")
            bneg = cpool.tile([128, 1], f32, tag="bneg")
            fout = cpool.tile([128, 512], f32, tag="fout")
            dumm = cpool.tile([128, 1], f32, tag="dumm")

            # persistent fina accumulator: 1 PSUM bank
            fps = fpool.tile([128, 512], f32, tag="fps")

            # K pad rows come zeroed from the host; 4-way split uses parallel
            # DMA queues so inputs land in ~3us
            nc.sync.dma_start(out=f1s[:, :], in_=f1p_d[:, :])
            for qd in range(4):
                nc.sync.dma_start(
                    out=f2s[32 * qd : 32 * qd + 32, :],
                    in_=f2p_d[32 * qd : 32 * qd + 32, :],
                )
            nc.sync.dma_start(out=vts[:, :], in_=vt_d[:, :])
            nc.gpsimd.memset(bneg[:, :], -5.0)
            # dummy activation: pulls the ~2.7us exp table load under the DMAs
            nc.scalar.activation(dumm[:, 0:1], bneg[:, 0:1], EXP)
            nc.gpsimd.memset(vtbA[:, :], 0.0)
            nc.gpsimd.memset(vtbB[:, :], 0.0)

            def emit_fina(pb):
                vtb = vtbA if pb % 2 == 0 else vtbB
                et = et_of[pb]
                for s in range(NSUB):
                    w = min(512, HW - 512 * s)
                    nc.tensor.matmul(
                        fps[0:64, 0:w],
                        lhsT=vtb[:, 48 - 4 * s : 112 - 4 * s],
                        rhs=et[:, 512 * s : 512 * s + w],
                        start=(pb == 0 and s == 0),
                        stop=(pb == PBLK - 1 and s == NSUB - 1),
                        skip_group_check=True,
                    )

            et_of = {}
            for pb in range(PBLK):
                et = epool.tile([128, HW], f16, tag="et")
                et_of[pb] = et
                zparts = zpool.tile([128, 4], f32, tag="zparts")
                rz = zpool.tile([128, 1], f32, tag="rz")

                for ci, (q0, width, reg) in enumerate(CHUNKS):
                    # fina(pb-1) emitted before the last S chunk: frees the
                    # PE to start S(pb+1) right after ACT(pb) region-A read
                    if ci == 3 and pb > 0:
                        emit_fina(pb - 1)
                    st = spool.tile([128, 2048 if reg == "A" else 1536], f32,
                                    tag="st" + reg)
                    for off in range(0, width, 512):
                        w = min(512, width - off)
                        # M=64 halves: 16-bit moving operand streams 2
                        # cols/cycle only when M <= 64 (PSUM drain port)
                        for h in range(2):
                            nc.tensor.matmul(
                                st[64 * h : 64 * h + 64, off : off + w],
                                lhsT=f1s[:, 128 * pb + 64 * h : 128 * pb + 64 * h + 64],
                                rhs=f2s[:, q0 + off : q0 + off + w],
                                start=True,
                                stop=True,
                                skip_group_check=True,
                            )
                    # bias -5: keeps exp within fp16 range (softmax is
                    # shift-invariant; Z accumulates the same shifted values)
                    nc.scalar.activation(
                        et[:, q0 : q0 + width],
                        st[:, 0:width],
                        EXP,
                        bias=bneg[:, 0:1],
                        accum_out=zparts[:, ci : ci + 1],
                    )

                # Z = sum of chunk partials; vtb cols 48:52 = V[:, block]/Z
                nc.vector.tensor_reduce(rz[:, 0:1], zparts[:, :], AXX, ADD)
                nc.vector.reciprocal(rz[:, 0:1], rz[:, 0:1])
                vtb = vtbA if pb % 2 == 0 else vtbB
                nc.vector.tensor_scalar(
                    vtb[:, 48:52],
                    vts[:, NCLS * pb : NCLS * pb + NCLS],
                    rz[:, 0:1],
                    2048.0,
                    MULT,
                    MULT,
                )

            emit_fina(PBLK - 1)
            nc.vector.tensor_copy(fout[:, :], fps[:, :])
            nc.sync.dma_start(out=res_d[:, :], in_=fout[0 : 4 * NSUB, :])

    nc.compile()
    return nc


def _get_nc():
    if "nc" not in _CACHE:
        _CACHE["nc"] = _build_bass()
    return _CACHE["nc"]


def _hilo16(x):
    """fp16 high/low split: x ~= hi + lo exactly to ~2^-22 relative."""
    x = np.asarray(x, np.float32)
    hi = x.astype(np.float16)
    lo = (x - hi.astype(np.float32)).astype(np.float16)
    return hi, lo


def _prep_inputs(feature_in, out, w1, b1, w2, b2):
    feature_in = np.asarray(feature_in, np.float32)
    out = np.asarray(out, np.float32)
    w1 = np.asarray(w1, np.float32)
    b1 = np.asarray(b1, np.float32)
    w2 = np.asarray(w2, np.float32)
    b2 = np.asarray(b2, np.float32)

    scale = np.float32(1.0 / np.sqrt(NCLS))
    feat = feature_in.reshape(NB, C_IN, HW)
    # f1 carries the softmax scale; f2 is plain
    f1 = (np.einsum("oc,ncp->nop", w1, feat, dtype=np.float32) + b1[None, :, None]) * scale
    f2 = np.einsum("oc,ncp->nop", w2, feat, dtype=np.float32) + b2[None, :, None]
    f1 = f1.astype(np.float32)
    f2 = f2.astype(np.float32)
    v = _resize_bilinear_ac(out, H, W).reshape(NB, NCLS, HW)

    in_maps = []
    for core in range(N_CORES):
        b, s = divmod(core, NSH)
        p0 = PSH * s
        f1p = np.zeros((128, PPAD), np.float16)
        h1, l1 = _hilo16(f1[b][:, p0 : p0 + PSH])
        f1p[0:4, :PSH] = h1
        f1p[4:8, :PSH] = l1
        f1p[8:12, :PSH] = h1
        h2, l2 = _hilo16(f2[b])
        f2p = np.zeros((128, HW), np.float16)
        f2p[0:12] = np.concatenate([h2, h2, l2], axis=0)
        vtp = np.zeros((NCLS, PPAD), np.float32)
        vtp[:, :PSH] = v[b][:, p0 : p0 + PSH]
        # vt[part, 4*pb + c] = V[c, p0 + 128*pb + part]
        vt = vtp.reshape(NCLS, PBLK, 128).transpose(2, 1, 0).reshape(128, PBLK * NCLS)
        in_maps.append(
            {
                "f1p": f1p,
                "f2p": np.ascontiguousarray(f2p),
                "vt": np.ascontiguousarray(vt),
            }
        )
    return in_maps


def _unpack(results):
    """results: list of 8 dicts with 'res' [52, 512] -> fina [2,4,80,80]."""
    fina = np.zeros((NB, NCLS, HW), np.float32)
    for core in range(N_CORES):
        b, _s = divmod(core, NSH)
        res = np.asarray(results[core]["res"], np.float32)  # [52, 512]
        for s in range(NSUB):
            w = min(512, HW - 512 * s)
            fina[b, :, 512 * s : 512 * s + w] += res[4 * s : 4 * s + 4, :w]
    fina *= np.float32(1.0 / 2048.0)
    return fina.reshape(NB, NCLS, H, W)


def run(inputs, trace=False):
    from concourse.bass_utils import run_bass_kernel_spmd

    nc = _get_nc()
    in_maps = _prep_inputs(**inputs)
    r = run_bass_kernel_spmd(nc, in_maps, list(range(N_CORES)), trace=trace)
    return _unpack(r.results), r.exec_time_ns


def kernel(feature_in, out, w1, b1, w2, b2):
    result, _ = run(
        dict(feature_in=feature_in, out=out, w1=w1, b1=b1, w2=w2, b2=b2)
    )
    return result


# revision 20
# speedup vs baseline: 1.3145x; 1.0315x over previous
"""Trainium2 Bass kernel for nn_Corr (correlation-attention module).

Math (per batch n):
    f1 = 0.5*(w1 @ feat + b1)        # [4, 6400]   feat = feature_in[n] flattened
    f2 =      w2 @ feat + b2         # [4, 6400]
    S  = f1^T @ f2                   # [6400, 6400]  (0.5 = 1/sqrt(nclass) folded into f1)
    A  = softmax(S, axis=1)          # row softmax (over q)
    V  = bilinear_resize(out[n])     # [4, 6400]
    fina[c, q] = sum_p V[c, p]/Z_p * exp(S[p, q])

Sharding: 2 batches x 4 p-shards (rows of S) = 8 cores. Each core produces a
partial fina over its 1600 p-rows; host sums the 4 partials per batch.

Device kernel per core, ScalarE(exp)-bound design (~6.6us per 128-row p-block):
  - S chunk = matmul(lhsT=f1pad[:, block], rhs=f2pad[:, qchunk]) with M=128,
    K=128 (12 live rows: fp16 hi/lo split), N=512 per MM (one PSUM bank).
  - Two ping-pong PSUM regions A=[128,2048] (4 banks) / B=[128,1536] (3 banks)
    give 4 wide EXP activations per block (2048/1536/2048/768), each with
    accum_out producing the row-sum partials Z for free.
  - fina accumulates IN PSUM (1 bank, [128,512]) across all 13 p-blocks:
    MM s (q-subgroup of 512) uses a zero-padded [128,128] lhsT slice of a
    sliding window buffer holding V/Z at cols 48:52, so output partitions
    4s..4s+4 catch class c of q-subgroup s and all other partitions get +0.
  - Emission order per iteration: S(pb) MMs+EXP, then fina(pb-1) MMs, then
    Z/vt scaling on VectorE -> PE never blocks the ACT stream.
"""

import numpy as np

N_CORES = 8
NB = 2          # batches
NCLS = 4        # nclass
C_IN = 32
H = W = 80
HW = H * W      # 6400
NSH = 4         # p-shards per batch
PSH = HW // NSH  # 1600 p rows per shard
PBLK = 13        # p blocks of 128 (1664 = 13*128, last 64 rows are zero-pad)
PPAD = PBLK * 128  # 1664
NSUB = 13        # fina q-subgroups of 512 (12*512 + 256)
# S chunk widths per block: ping-pong regions A (4 banks) / B (3 banks)
CHUNKS = ((0, 2048, "A"), (2048, 1536, "B"), (3584, 2048, "A"), (5632, 768, "B"))

_CACHE = {}


def _resize_bilinear_ac(x, h_out, w_out):
    """numpy mirror of the reference's align_corners=True bilinear resize."""
    n, c, h, w = x.shape
    if (h, w) == (h_out, w_out):
        return x
    ys = np.linspace(0.0, h - 1.0, h_out, dtype=np.float32)
    xs = np.linspace(0.0, w - 1.0, w_out, dtype=np.float32)
    y0 = np.floor(ys).astype(np.int32)
    x0 = np.floor(xs).astype(np.int32)
    y1 = np.minimum(y0 + 1, h - 1)
    x1 = np.minimum(x0 + 1, w - 1)
    wy = (ys - y0.astype(np.float32))[None, None, :, None]
    wx = (xs - x0.astype(np.float32))[None, None, None, :]
    g = lambda yi, xi: x[:, :, yi, :][:, :, :, xi]
    top = g(y0, x0) * (1.0 - wx) + g(y0, x1) * wx
    bot = g(y1, x0) * (1.0 - wx) + g(y1, x1) * wx
    return (top * (1.0 - wy) + bot * wy).astype(np.float32)


def _build_bass():
    import concourse.bass as bass
    import concourse.tile as tile
    from concourse import bacc, mybir

    f32 = mybir.dt.float32
    f16 = mybir.dt.float16

    nc = bacc.Bacc(
        "TRN2", target_bir_lowering=False, debug=False, num_devices=N_CORES
    )

    f1p_d = nc.dram_tensor("f1p", [32, PPAD], f16, kind="ExternalInput")
    f2p_d = nc.dram_tensor("f2p", [32, HW], f16, kind="ExternalInput")
    zz_d = nc.dram_tensor("zz", [32, HW], f16, kind="ExternalInput")
    vt_d = nc.dram_tensor("vt", [128, NCLS * PBLK], f32, kind="ExternalInput")
    res_d = nc.dram_tensor("res", [4 * NSUB, 512], f32, kind="ExternalOutput")

    EXP = mybir.ActivationFunctionType.Exp
    ADD = mybir.AluOpType.add
    MULT = mybir.AluOpType.mult
    AXX = mybir.AxisListType.X

    with tile.TileContext(nc) as tc:
        with (
            tc.tile_pool(name="const", bufs=1) as cpool,
            tc.tile_pool(name="estrip", bufs=2) as epool,
            tc.tile_pool(name="zpool", bufs=2) as zpool,
            tc.tile_pool(name="spsum", bufs=1, space="PSUM") as spool,
            tc.tile_pool(name="fpsum", bufs=1, space="PSUM") as fpool,
        ):
            f1s = cpool.tile([128, PPAD], f16, tag="f1s")
            f2s = cpool.tile([128, HW], f16, tag="f2s")
            vts = cpool.tile([128, NCLS * PBLK], f32, tag="vts")
            # sliding-window fina weights: V/Z at cols 48:52, zeros elsewhere;
            # MM s uses the [128,64] slice starting at col 48-4s
            vtbA = cpool.tile([128, 112], f16, tag="vtbA")
            vtbB = cpool.tile([128, 112], f16, tag="vtbB")
            bneg = cpool.tile([128, 1], f32, tag="bneg")
            fout = cpool.tile([128, 512], f32, tag="fout")
            dumm = cpool.tile([128, 1], f32, tag="dumm")

            # persistent fina accumulator: 1 PSUM bank
            fps = fpool.tile([128, 512], f32, tag="fps")

            # zeroing the K pad rows (12:128) is spread over DMA (host zeros),
            # GpSimd, and DVE so the head is limited only by the live-row DMAs
            u32 = mybir.dt.uint32
            nc.sync.dma_start(out=f1s[0:32, :], in_=f1p_d[:, :])
            nc.sync.dma_start(out=f2s[0:32, :], in_=f2p_d[:, :])
            nc.sync.dma_start(out=f2s[96:128, :], in_=zz_d[:, :])
            nc.sync.dma_start(out=vts[:, :], in_=vt_d[:, :])
            nc.gpsimd.memset(bneg[:, :], -5.0)
            # dummy activation: pulls the ~2.7us exp table load under the DMAs
            nc.scalar.activation(dumm[:, 0:1], bneg[:, 0:1], EXP)
            nc.gpsimd.memset(vtbA[:, :], 0.0)
            nc.gpsimd.memset(vtbB[:, :], 0.0)
            nc.gpsimd.memset(f1s[32:64, :], 0.0)
            nc.gpsimd.memset(f1s[64:96, :], 0.0)
            nc.gpsimd.memset(f1s[96:128, :], 0.0)
            nc.gpsimd.memset(f2s[32:64, :], 0.0)
            nc.vector.memset(f2s[64:96, :].bitcast(u32), 0)
            # warm-up matmuls on zeroed tiles: ~4us of PE activity flips the
            # HAM clock gate to 8/8 while the input DMAs are still in flight
            wst = spool.tile([128, 2048], f32, tag="stA")
            for _ in range(16):
                nc.tensor.matmul(
                    wst[0:64, 0:112],
                    lhsT=vtbB[:, 0:64],
                    rhs=vtbA[:, 0:112],
                    start=True,
                    stop=True,
                    skip_group_check=True,
                )

            def emit_fina(pb):
                vtb = vtbA if pb % 2 == 0 else vtbB
                et = et_of[pb]
                for s in range(NSUB):
                    w = min(512, HW - 512 * s)
                    nc.tensor.matmul(
                        fps[0:64, 0:w],
                        lhsT=vtb[:, 48 - 4 * s : 112 - 4 * s],
                        rhs=et[:, 512 * s : 512 * s + w],
                        start=(pb == 0 and s == 0),
                        stop=(pb == PBLK - 1 and s == NSUB - 1),
                        skip_group_check=True,
                    )

            et_of = {}
            for pb in range(PBLK):
                et = epool.tile([128, HW], f16, tag="et")
                et_of[pb] = et
                zparts = zpool.tile([128, 4], f32, tag="zparts")
                rz = zpool.tile([128, 1], f32, tag="rz")

                for ci, (q0, width, reg) in enumerate(CHUNKS):
                    # fina(pb-1) emitted before the last S chunk: frees the
                    # PE to start S(pb+1) right after ACT(pb) region-A read
                    if ci == 3 and pb > 0:
                        emit_fina(pb - 1)
                    st = spool.tile([128, 2048 if reg == "A" else 1536], f32,
                                    tag="st" + reg)
                    for off in range(0, width, 512):
                        w = min(512, width - off)
                        # M=64 halves: 16-bit moving operand streams 2
                        # cols/cycle only when M <= 64 (PSUM drain port)
                        for h in range(2):
                            nc.tensor.matmul(
                                st[64 * h : 64 * h + 64, off : off + w],
                                lhsT=f1s[:, 128 * pb + 64 * h : 128 * pb + 64 * h + 64],
                                rhs=f2s[:, q0 + off : q0 + off + w],
                                start=True,
                                stop=True,
                                skip_group_check=True,
                            )
                    # bias -5: keeps exp within fp16 range (softmax is
                    # shift-invariant; Z accumulates the same shifted values)
                    nc.scalar.activation(
                        et[:, q0 : q0 + width],
                        st[:, 0:width],
                        EXP,
                        bias=bneg[:, 0:1],
                        accum_out=zparts[:, ci : ci + 1],
                    )

                # Z = sum of chunk partials; vtb cols 48:52 = V[:, block]/Z
                nc.vector.tensor_reduce(rz[:, 0:1], zparts[:, :], AXX, ADD)
                nc.vector.reciprocal(rz[:, 0:1], rz[:, 0:1])
                vtb = vtbA if pb % 2 == 0 else vtbB
                nc.vector.tensor_scalar(
                    vtb[:, 48:52],
                    vts[:, NCLS * pb : NCLS * pb + NCLS],
                    rz[:, 0:1],
                    2048.0,
                    MULT,
                    MULT,
                )

            emit_fina(PBLK - 1)
            nc.vector.tensor_copy(fout[:, :], fps[:, :])
            nc.sync.dma_start(out=res_d[:, :], in_=fout[0 : 4 * NSUB, :])

    nc.compile()
    return nc


def _get_nc():
    if "nc" not in _CACHE:
        _CACHE["nc"] = _build_bass()
    return _CACHE["nc"]


def _hilo16(x):
    """fp16 high/low split: x ~= hi + lo exactly to ~2^-22 relative."""
    x = np.asarray(x, np.float32)
    hi = x.astype(np.float16)
    lo = (x - hi.astype(np.float32)).astype(np.float16)
    return hi, lo


def _prep_inputs(feature_in, out, w1, b1, w2, b2):
    feature_in = np.asarray(feature_in, np.float32)
    out = np.asarray(out, np.float32)
    w1 = np.asarray(w1, np.float32)
    b1 = np.asarray(b1, np.float32)
    w2 = np.asarray(w2, np.float32)
    b2 = np.asarray(b2, np.float32)

    scale = np.float32(1.0 / np.sqrt(NCLS))
    feat = feature_in.reshape(NB, C_IN, HW)
    # f1 carries the softmax scale; f2 is plain
    f1 = (np.einsum("oc,ncp->nop", w1, feat, dtype=np.float32) + b1[None, :, None]) * scale
    f2 = np.einsum("oc,ncp->nop", w2, feat, dtype=np.float32) + b2[None, :, None]
    f1 = f1.astype(np.float32)
    f2 = f2.astype(np.float32)
    v = _resize_bilinear_ac(out, H, W).reshape(NB, NCLS, HW)

    in_maps = []
    for core in range(N_CORES):
        b, s = divmod(core, NSH)
        p0 = PSH * s
        f1p = np.zeros((32, PPAD), np.float16)
        h1, l1 = _hilo16(f1[b][:, p0 : p0 + PSH])
        f1p[0:4, :PSH] = h1
        f1p[4:8, :PSH] = l1
        f1p[8:12, :PSH] = h1
        h2, l2 = _hilo16(f2[b])
        f2p = np.zeros((32, HW), np.float16)
        f2p[0:12] = np.concatenate([h2, h2, l2], axis=0)
        vtp = np.zeros((NCLS, PPAD), np.float32)
        vtp[:, :PSH] = v[b][:, p0 : p0 + PSH]
        # vt[part, 4*pb + c] = V[c, p0 + 128*pb + part]
        vt = vtp.reshape(NCLS, PBLK, 128).transpose(2, 1, 0).reshape(128, PBLK * NCLS)
        in_maps.append(
            {
                "f1p": f1p,
                "f2p": np.ascontiguousarray(f2p),
                "vt": np.ascontiguousarray(vt),
                "zz": np.zeros((32, HW), np.float16),
            }
        )
    return in_maps


def _unpack(results):
    """results: list of 8 dicts with 'res' [52, 512] -> fina [2,4,80,80]."""
    fina = np.zeros((NB, NCLS, HW), np.float32)
    for core in range(N_CORES):
        b, _s = divmod(core, NSH)
        res = np.asarray(results[core]["res"], np.float32)  # [52, 512]
        for s in range(NSUB):
            w = min(512, HW - 512 * s)
            fina[b, :, 512 * s : 512 * s + w] += res[4 * s : 4 * s + 4, :w]
    fina *= np.float32(1.0 / 2048.0)
    return fina.reshape(NB, NCLS, H, W)


def run(inputs, trace=False):
    from concourse.bass_utils import run_bass_kernel_spmd

    nc = _get_nc()
    in_maps = _prep_inputs(**inputs)
    r = run_bass_kernel_spmd(nc, in_maps, list(range(N_CORES)), trace=trace)
    return _unpack(r.results), r.exec_time_ns


def kernel(feature_in, out, w1, b1, w2, b2):
    result, _ = run(
        dict(feature_in=feature_in, out=out, w1=w1, b1=b1, w2=w2, b2=b2)
    )
    return result


# revision 21
# speedup vs baseline: 1.3165x; 1.0015x over previous
"""Trainium2 Bass kernel for nn_Corr (correlation-attention module).

Math (per batch n):
    f1 = 0.5*(w1 @ feat + b1)        # [4, 6400]   feat = feature_in[n] flattened
    f2 =      w2 @ feat + b2         # [4, 6400]
    S  = f1^T @ f2                   # [6400, 6400]  (0.5 = 1/sqrt(nclass) folded into f1)
    A  = softmax(S, axis=1)          # row softmax (over q)
    V  = bilinear_resize(out[n])     # [4, 6400]
    fina[c, q] = sum_p V[c, p]/Z_p * exp(S[p, q])

Sharding: 2 batches x 4 p-shards (rows of S) = 8 cores. Each core produces a
partial fina over its 1600 p-rows; host sums the 4 partials per batch.

Device kernel per core, ScalarE(exp)-bound design (~6.6us per 128-row p-block):
  - S chunk = matmul(lhsT=f1pad[:, block], rhs=f2pad[:, qchunk]) with M=128,
    K=128 (12 live rows: fp16 hi/lo split), N=512 per MM (one PSUM bank).
  - Two ping-pong PSUM regions A=[128,2048] (4 banks) / B=[128,1536] (3 banks)
    give 4 wide EXP activations per block (2048/1536/2048/768), each with
    accum_out producing the row-sum partials Z for free.
  - fina accumulates IN PSUM (1 bank, [128,512]) across all 13 p-blocks:
    MM s (q-subgroup of 512) uses a zero-padded [128,128] lhsT slice of a
    sliding window buffer holding V/Z at cols 48:52, so output partitions
    4s..4s+4 catch class c of q-subgroup s and all other partitions get +0.
  - Emission order per iteration: S(pb) MMs+EXP, then fina(pb-1) MMs, then
    Z/vt scaling on VectorE -> PE never blocks the ACT stream.
"""

import numpy as np

N_CORES = 8
NB = 2          # batches
NCLS = 4        # nclass
C_IN = 32
H = W = 80
HW = H * W      # 6400
NSH = 4         # p-shards per batch
PSH = HW // NSH  # 1600 p rows per shard
PBLK = 13        # p blocks of 128 (1664 = 13*128, last 64 rows are zero-pad)
PPAD = PBLK * 128  # 1664
NSUB = 13        # fina q-subgroups of 512 (12*512 + 256)
# S chunk widths per block: ping-pong regions A (4 banks) / B (3 banks)
CHUNKS = ((0, 2048, "A"), (2048, 1536, "B"), (3584, 2048, "A"), (5632, 768, "B"))

_CACHE = {}


def _resize_bilinear_ac(x, h_out, w_out):
    """numpy mirror of the reference's align_corners=True bilinear resize."""
    n, c, h, w = x.shape
    if (h, w) == (h_out, w_out):
        return x
    ys = np.linspace(0.0, h - 1.0, h_out, dtype=np.float32)
    xs = np.linspace(0.0, w - 1.0, w_out, dtype=np.float32)
    y0 = np.floor(ys).astype(np.int32)
    x0 = np.floor(xs).astype(np.int32)
    y1 = np.minimum(y0 + 1, h - 1)
    x1 = np.minimum(x0 + 1, w - 1)
    wy = (ys - y0.astype(np.float32))[None, None, :, None]
    wx = (xs - x0.astype(np.float32))[None, None, None, :]
    g = lambda yi, xi: x[:, :, yi, :][:, :, :, xi]
    top = g(y0, x0) * (1.0 - wx) + g(y0, x1) * wx
    bot = g(y1, x0) * (1.0 - wx) + g(y1, x1) * wx
    return (top * (1.0 - wy) + bot * wy).astype(np.float32)


def _build_bass():
    import concourse.bass as bass
    import concourse.tile as tile
    from concourse import bacc, mybir

    f32 = mybir.dt.float32
    f16 = mybir.dt.float16

    nc = bacc.Bacc(
        "TRN2", target_bir_lowering=False, debug=False, num_devices=N_CORES
    )

    f1p_d = nc.dram_tensor("f1p", [32, PPAD], f16, kind="ExternalInput")
    f2p_d = nc.dram_tensor("f2p", [32, HW], f16, kind="ExternalInput")
    zz_d = nc.dram_tensor("zz", [32, HW], f16, kind="ExternalInput")
    vt_d = nc.dram_tensor("vt", [128, NCLS * PBLK], f32, kind="ExternalInput")
    res_d = nc.dram_tensor("res", [4 * NSUB, 512], f32, kind="ExternalOutput")

    EXP = mybir.ActivationFunctionType.Exp
    ADD = mybir.AluOpType.add
    MULT = mybir.AluOpType.mult
    AXX = mybir.AxisListType.X

    with tile.TileContext(nc) as tc:
        with (
            tc.tile_pool(name="const", bufs=1) as cpool,
            tc.tile_pool(name="estrip", bufs=2) as epool,
            tc.tile_pool(name="zpool", bufs=2) as zpool,
            tc.tile_pool(name="spsum", bufs=1, space="PSUM") as spool,
            tc.tile_pool(name="fpsum", bufs=1, space="PSUM") as fpool,
        ):
            f1s = cpool.tile([128, PPAD], f16, tag="f1s")
            f2s = cpool.tile([128, HW], f16, tag="f2s")
            vts = cpool.tile([128, NCLS * PBLK], f32, tag="vts")
            # sliding-window fina weights: V/Z at cols 48:52, zeros elsewhere;
            # MM s uses the [128,64] slice starting at col 48-4s
            vtbA = cpool.tile([128, 112], f16, tag="vtbA")
            vtbB = cpool.tile([128, 112], f16, tag="vtbB")
            bneg = cpool.tile([128, 1], f32, tag="bneg")
            fout = cpool.tile([128, 512], f32, tag="fout")
            dumm = cpool.tile([128, 1], f32, tag="dumm")

            # persistent fina accumulator: 1 PSUM bank
            fps = fpool.tile([128, 512], f32, tag="fps")

            # zeroing the K pad rows (12:128) is spread over DMA (host zeros),
            # GpSimd, and DVE so the head is limited only by the live-row DMAs
            u32 = mybir.dt.uint32
            nc.sync.dma_start(out=f1s[0:32, :], in_=f1p_d[:, :])
            nc.sync.dma_start(out=f2s[0:32, :], in_=f2p_d[:, :])
            nc.sync.dma_start(out=f2s[96:128, :], in_=zz_d[:, :])
            nc.sync.dma_start(out=vts[:, :], in_=vt_d[:, :])
            nc.gpsimd.memset(bneg[:, :], -5.0)
            # dummy activation: pulls the ~2.7us exp table load under the DMAs
            nc.scalar.activation(dumm[:, 0:1], bneg[:, 0:1], EXP)
            nc.sync.dma_start(out=f1s[96:128, :], in_=zz_d[:, 0:PPAD])
            nc.gpsimd.memset(vtbA[:, :], 0.0)
            nc.gpsimd.memset(vtbB[:, :], 0.0)
            nc.gpsimd.memset(f1s[32:64, :], 0.0)
            nc.gpsimd.memset(f1s[64:96, :], 0.0)
            nc.vector.memset(f2s[32:64, :].bitcast(u32), 0)
            nc.vector.memset(f2s[64:96, :].bitcast(u32), 0)
            # warm-up matmuls on zeroed tiles: PE activity until the input
            # DMAs land keeps the HAM clock gate at 8/8 for the real matmuls
            wst = spool.tile([128, 2048], f32, tag="stA")
            for _ in range(40):
                nc.tensor.matmul(
                    wst[0:64, 0:112],
                    lhsT=vtbB[:, 0:64],
                    rhs=vtbA[:, 0:112],
                    start=True,
                    stop=True,
                    skip_group_check=True,
                )

            def emit_fina(pb):
                vtb = vtbA if pb % 2 == 0 else vtbB
                et = et_of[pb]
                for s in range(NSUB):
                    w = min(512, HW - 512 * s)
                    nc.tensor.matmul(
                        fps[0:64, 0:w],
                        lhsT=vtb[:, 48 - 4 * s : 112 - 4 * s],
                        rhs=et[:, 512 * s : 512 * s + w],
                        start=(pb == 0 and s == 0),
                        stop=(pb == PBLK - 1 and s == NSUB - 1),
                        skip_group_check=True,
                    )

            et_of = {}
            for pb in range(PBLK):
                et = epool.tile([128, HW], f16, tag="et")
                et_of[pb] = et
                zparts = zpool.tile([128, 4], f32, tag="zparts")
                rz = zpool.tile([128, 1], f32, tag="rz")

                for ci, (q0, width, reg) in enumerate(CHUNKS):
                    # fina(pb-1) emitted before the last S chunk: frees the
                    # PE to start S(pb+1) right after ACT(pb) region-A read
                    if ci == 3 and pb > 0:
                        emit_fina(pb - 1)
                    st = spool.tile([128, 2048 if reg == "A" else 1536], f32,
                                    tag="st" + reg)
                    for off in range(0, width, 512):
                        w = min(512, width - off)
                        # M=64 halves: 16-bit moving operand streams 2
                        # cols/cycle only when M <= 64 (PSUM drain port)
                        for h in range(2):
                            nc.tensor.matmul(
                                st[64 * h : 64 * h + 64, off : off + w],
                                lhsT=f1s[:, 128 * pb + 64 * h : 128 * pb + 64 * h + 64],
                                rhs=f2s[:, q0 + off : q0 + off + w],
                                start=True,
                                stop=True,
                                skip_group_check=True,
                            )
                    # bias -5: keeps exp within fp16 range (softmax is
                    # shift-invariant; Z accumulates the same shifted values)
                    nc.scalar.activation(
                        et[:, q0 : q0 + width],
                        st[:, 0:width],
                        EXP,
                        bias=bneg[:, 0:1],
                        accum_out=zparts[:, ci : ci + 1],
                    )

                # Z = sum of chunk partials; vtb cols 48:52 = V[:, block]/Z
                nc.vector.tensor_reduce(rz[:, 0:1], zparts[:, :], AXX, ADD)
                nc.vector.reciprocal(rz[:, 0:1], rz[:, 0:1])
                vtb = vtbA if pb % 2 == 0 else vtbB
                nc.vector.tensor_scalar(
                    vtb[:, 48:52],
                    vts[:, NCLS * pb : NCLS * pb + NCLS],
                    rz[:, 0:1],
                    2048.0,
                    MULT,
                    MULT,
                )

            emit_fina(PBLK - 1)
            nc.vector.tensor_copy(fout[:, :], fps[:, :])
            nc.sync.dma_start(out=res_d[:, :], in_=fout[0 : 4 * NSUB, :])

    nc.compile()
    return nc


def _get_nc():
    if "nc" not in _CACHE:
        _CACHE["nc"] = _build_bass()
    return _CACHE["nc"]


def _hilo16(x):
    """fp16 high/low split: x ~= hi + lo exactly to ~2^-22 relative."""
    x = np.asarray(x, np.float32)
    hi = x.astype(np.float16)
    lo = (x - hi.astype(np.float32)).astype(np.float16)
    return hi, lo


def _prep_inputs(feature_in, out, w1, b1, w2, b2):
    feature_in = np.asarray(feature_in, np.float32)
    out = np.asarray(out, np.float32)
    w1 = np.asarray(w1, np.float32)
    b1 = np.asarray(b1, np.float32)
    w2 = np.asarray(w2, np.float32)
    b2 = np.asarray(b2, np.float32)

    scale = np.float32(1.0 / np.sqrt(NCLS))
    feat = feature_in.reshape(NB, C_IN, HW)
    # f1 carries the softmax scale; f2 is plain
    f1 = (np.einsum("oc,ncp->nop", w1, feat, dtype=np.float32) + b1[None, :, None]) * scale
    f2 = np.einsum("oc,ncp->nop", w2, feat, dtype=np.float32) + b2[None, :, None]
    f1 = f1.astype(np.float32)
    f2 = f2.astype(np.float32)
    v = _resize_bilinear_ac(out, H, W).reshape(NB, NCLS, HW)

    in_maps = []
    for core in range(N_CORES):
        b, s = divmod(core, NSH)
        p0 = PSH * s
        f1p = np.zeros((32, PPAD), np.float16)
        h1, l1 = _hilo16(f1[b][:, p0 : p0 + PSH])
        f1p[0:4, :PSH] = h1
        f1p[4:8, :PSH] = l1
        f1p[8:12, :PSH] = h1
        h2, l2 = _hilo16(f2[b])
        f2p = np.zeros((32, HW), np.float16)
        f2p[0:12] = np.concatenate([h2, h2, l2], axis=0)
        vtp = np.zeros((NCLS, PPAD), np.float32)
        vtp[:, :PSH] = v[b][:, p0 : p0 + PSH]
        # vt[part, 4*pb + c] = V[c, p0 + 128*pb + part]
        vt = vtp.reshape(NCLS, PBLK, 128).transpose(2, 1, 0).reshape(128, PBLK * NCLS)
        in_maps.append(
            {
                "f1p": f1p,
                "f2p": np.ascontiguousarray(f2p),
                "vt": np.ascontiguousarray(vt),
                "zz": np.zeros((32, HW), np.float16),
            }
        )
    return in_maps


def _unpack(results):
    """results: list of 8 dicts with 'res' [52, 512] -> fina [2,4,80,80]."""
    fina = np.zeros((NB, NCLS, HW), np.float32)
    for core in range(N_CORES):
        b, _s = divmod(core, NSH)
        res = np.asarray(results[core]["res"], np.float32)  # [52, 512]
        for s in range(NSUB):
            w = min(512, HW - 512 * s)
            fina[b, :, 512 * s : 512 * s + w] += res[4 * s : 4 * s + 4, :w]
    fina *= np.float32(1.0 / 2048.0)
    return fina.reshape(NB, NCLS, H, W)


def run(inputs, trace=False):
    from concourse.bass_utils import run_bass_kernel_spmd

    nc = _get_nc()
    in_maps = _prep_inputs(**inputs)
    r = run_bass_kernel_spmd(nc, in_maps, list(range(N_CORES)), trace=trace)
    return _unpack(r.results), r.exec_time_ns


def kernel(feature_in, out, w1, b1, w2, b2):
    result, _ = run(
        dict(feature_in=feature_in, out=out, w1=w1, b1=b1, w2=w2, b2=b2)
    )
    return result


# revision 22
# speedup vs baseline: 1.3401x; 1.0180x over previous
"""Trainium2 Bass kernel for nn_Corr (correlation-attention module).

Math (per batch n):
    f1 = 0.5*(w1 @ feat + b1)        # [4, 6400]   feat = feature_in[n] flattened
    f2 =      w2 @ feat + b2         # [4, 6400]
    S  = f1^T @ f2                   # [6400, 6400]  (0.5 = 1/sqrt(nclass) folded into f1)
    A  = softmax(S, axis=1)          # row softmax (over q)
    V  = bilinear_resize(out[n])     # [4, 6400]
    fina[c, q] = sum_p V[c, p]/Z_p * exp(S[p, q])

Sharding: 2 batches x 4 p-shards (rows of S) = 8 cores. Each core produces a
partial fina over its 1600 p-rows; host sums the 4 partials per batch.

Device kernel per core, ScalarE(exp)-bound design (~6.6us per 128-row p-block):
  - S chunk = matmul(lhsT=f1pad[:, block], rhs=f2pad[:, qchunk]) with M=128,
    K=128 (12 live rows: fp16 hi/lo split), N=512 per MM (one PSUM bank).
  - Two ping-pong PSUM regions A=[128,2048] (4 banks) / B=[128,1536] (3 banks)
    give 4 wide EXP activations per block (2048/1536/2048/768), each with
    accum_out producing the row-sum partials Z for free.
  - fina accumulates IN PSUM (1 bank, [128,512]) across all 13 p-blocks:
    MM s (q-subgroup of 512) uses a zero-padded [128,128] lhsT slice of a
    sliding window buffer holding V/Z at cols 48:52, so output partitions
    4s..4s+4 catch class c of q-subgroup s and all other partitions get +0.
  - Emission order per iteration: S(pb) MMs+EXP, then fina(pb-1) MMs, then
    Z/vt scaling on VectorE -> PE never blocks the ACT stream.
"""

import numpy as np

N_CORES = 8
NB = 2          # batches
NCLS = 4        # nclass
C_IN = 32
H = W = 80
HW = H * W      # 6400
NSH = 4         # p-shards per batch
PSH = HW // NSH  # 1600 p rows per shard
PBLK = 13        # p blocks of 128 (1664 = 13*128, last 64 rows are zero-pad)
PPAD = PBLK * 128  # 1664
NSUB = 13        # fina q-subgroups of 512 (12*512 + 256)
# S chunk widths per block: ping-pong regions A (4 banks) / B (3 banks)
CHUNKS = ((0, 2048, "A"), (2048, 1536, "B"), (3584, 2048, "A"), (5632, 768, "B"))

_CACHE = {}


def _resize_bilinear_ac(x, h_out, w_out):
    """numpy mirror of the reference's align_corners=True bilinear resize."""
    n, c, h, w = x.shape
    if (h, w) == (h_out, w_out):
        return x
    ys = np.linspace(0.0, h - 1.0, h_out, dtype=np.float32)
    xs = np.linspace(0.0, w - 1.0, w_out, dtype=np.float32)
    y0 = np.floor(ys).astype(np.int32)
    x0 = np.floor(xs).astype(np.int32)
    y1 = np.minimum(y0 + 1, h - 1)
    x1 = np.minimum(x0 + 1, w - 1)
    wy = (ys - y0.astype(np.float32))[None, None, :, None]
    wx = (xs - x0.astype(np.float32))[None, None, None, :]
    g = lambda yi, xi: x[:, :, yi, :][:, :, :, xi]
    top = g(y0, x0) * (1.0 - wx) + g(y0, x1) * wx
    bot = g(y1, x0) * (1.0 - wx) + g(y1, x1) * wx
    return (top * (1.0 - wy) + bot * wy).astype(np.float32)


def _build_bass():
    import concourse.bass as bass
    import concourse.tile as tile
    from concourse import bacc, mybir

    f32 = mybir.dt.float32
    f16 = mybir.dt.float16

    nc = bacc.Bacc(
        "TRN2", target_bir_lowering=False, debug=False, num_devices=N_CORES
    )

    f1p_d = nc.dram_tensor("f1p", [32, PPAD], f16, kind="ExternalInput")
    f2p_d = nc.dram_tensor("f2p", [32, HW], f16, kind="ExternalInput")
    zz_d = nc.dram_tensor("zz", [32, HW], f16, kind="ExternalInput")
    vt_d = nc.dram_tensor("vt", [128, NCLS * PBLK], f32, kind="ExternalInput")
    res_d = nc.dram_tensor("res", [4 * NSUB, 512], f32, kind="ExternalOutput")

    EXP = mybir.ActivationFunctionType.Exp
    ADD = mybir.AluOpType.add
    MULT = mybir.AluOpType.mult
    AXX = mybir.AxisListType.X

    with tile.TileContext(nc) as tc:
        with (
            tc.tile_pool(name="const", bufs=1) as cpool,
            tc.tile_pool(name="estrip", bufs=2) as epool,
            tc.tile_pool(name="zpool", bufs=2) as zpool,
            tc.tile_pool(name="spsum", bufs=1, space="PSUM") as spool,
            tc.tile_pool(name="fpsum", bufs=1, space="PSUM") as fpool,
        ):
            f1s = cpool.tile([128, PPAD], f16, tag="f1s")
            f2s = cpool.tile([128, HW], f16, tag="f2s")
            vts = cpool.tile([128, NCLS * PBLK], f32, tag="vts")
            # sliding-window fina weights: V/Z at cols 48:52, zeros elsewhere;
            # MM s uses the [128,64] slice starting at col 48-4s
            vtbA = cpool.tile([128, 112], f16, tag="vtbA")
            vtbB = cpool.tile([128, 112], f16, tag="vtbB")
            bneg = cpool.tile([128, 1], f32, tag="bneg")
            fout = cpool.tile([128, 512], f32, tag="fout")
            dumm = cpool.tile([128, 1], f32, tag="dumm")

            # persistent fina accumulator: 1 PSUM bank
            fps = fpool.tile([128, 512], f32, tag="fps")

            # zeroing the K pad rows (12:128) is spread over DMA (host zeros),
            # GpSimd, and DVE so the head is limited only by the live-row DMAs
            u32 = mybir.dt.uint32
            nc.sync.dma_start(out=f1s[0:32, :], in_=f1p_d[:, :])
            nc.sync.dma_start(out=f2s[0:32, :], in_=f2p_d[:, :])
            nc.sync.dma_start(out=f2s[96:128, :], in_=zz_d[:, :])
            nc.sync.dma_start(out=vts[:, :], in_=vt_d[:, :])
            nc.gpsimd.memset(bneg[:, :], -5.0)
            # dummy activation: pulls the ~2.7us exp table load under the DMAs
            nc.scalar.activation(dumm[:, 0:1], bneg[:, 0:1], EXP)
            nc.sync.dma_start(out=f1s[96:128, :], in_=zz_d[:, 0:PPAD])
            nc.gpsimd.memset(vtbA[:, :], 0.0)
            nc.gpsimd.memset(vtbB[:, :], 0.0)
            nc.gpsimd.memset(f1s[32:64, :], 0.0)
            nc.gpsimd.memset(f1s[64:96, :], 0.0)
            nc.vector.memset(f2s[32:64, :].bitcast(u32), 0)
            nc.vector.memset(f2s[64:96, :].bitcast(u32), 0)
            # warm-up matmuls on zeroed tiles: PE activity until the input
            # DMAs land keeps the HAM clock gate at 8/8 for the real matmuls
            wst = spool.tile([128, 2048], f32, tag="stA")
            for _ in range(40):
                nc.tensor.matmul(
                    wst[0:64, 0:112],
                    lhsT=vtbB[:, 0:64],
                    rhs=vtbA[:, 0:112],
                    start=True,
                    stop=True,
                    skip_group_check=True,
                )

            def emit_fina(pb):
                vtb = vtbA if pb % 2 == 0 else vtbB
                et = et_of[pb]
                for s in range(NSUB):
                    w = min(512, HW - 512 * s)
                    nc.tensor.matmul(
                        fps[0:64, 0:w],
                        lhsT=vtb[:, 48 - 4 * s : 112 - 4 * s],
                        rhs=et[:, 512 * s : 512 * s + w],
                        start=(pb == 0 and s == 0),
                        stop=(pb == PBLK - 1 and s == NSUB - 1),
                        skip_group_check=True,
                    )

            et_of = {}
            for pb in range(PBLK):
                et = epool.tile([128, HW], f16, tag="et")
                et_of[pb] = et
                zparts = zpool.tile([128, 4], f32, tag="zparts")
                rz = zpool.tile([128, 1], f32, tag="rz")

                for ci, (q0, width, reg) in enumerate(CHUNKS):
                    # fina(pb-1) emitted before the last S chunk: frees the
                    # PE to start S(pb+1) right after ACT(pb) region-A read
                    if ci == 3 and pb > 0:
                        emit_fina(pb - 1)
                    st = spool.tile([128, 2048 if reg == "A" else 1536], f32,
                                    tag="st" + reg)
                    for off in range(0, width, 512):
                        w = min(512, width - off)
                        # M=64 halves: 16-bit moving operand streams 2
                        # cols/cycle only when M <= 64 (PSUM drain port)
                        for h in range(2):
                            nc.tensor.matmul(
                                st[64 * h : 64 * h + 64, off : off + w],
                                lhsT=f1s[:, 128 * pb + 64 * h : 128 * pb + 64 * h + 64],
                                rhs=f2s[:, q0 + off : q0 + off + w],
                                start=True,
                                stop=True,
                                skip_group_check=True,
                            )
                    # bias -5: keeps exp within fp16 range (softmax is
                    # shift-invariant; Z accumulates the same shifted values)
                    nc.scalar.activation(
                        et[:, q0 : q0 + width],
                        st[:, 0:width],
                        EXP,
                        bias=bneg[:, 0:1],
                        accum_out=zparts[:, ci : ci + 1],
                    )

                # Z = sum of chunk partials; vtb cols 48:52 = V[:, block]/Z
                nc.vector.tensor_reduce(rz[:, 0:1], zparts[:, :], AXX, ADD)
                nc.vector.reciprocal(rz[:, 0:1], rz[:, 0:1])
                vtb = vtbA if pb % 2 == 0 else vtbB
                nc.vector.tensor_scalar(
                    vtb[:, 48:52],
                    vts[:, NCLS * pb : NCLS * pb + NCLS],
                    rz[:, 0:1],
                    2048.0,
                    MULT,
                    MULT,
                )

            # keep the PE warm through the last ACT window (no S(pb+1) work
            # exists) so fina(12) runs at full clock instead of HAM-throttled
            wtl = spool.tile([128, 2048], f32, tag="stA")
            for _ in range(10):
                nc.tensor.matmul(
                    wtl[0:64, 0:512],
                    lhsT=vtbB[:, 0:64],
                    rhs=f2s[:, 0:512],
                    start=True,
                    stop=True,
                    skip_group_check=True,
                )
            emit_fina(PBLK - 1)
            nc.vector.tensor_copy(fout[:, :], fps[:, :])
            nc.sync.dma_start(out=res_d[:, :], in_=fout[0 : 4 * NSUB, :])

    nc.compile()
    return nc


def _get_nc():
    if "nc" not in _CACHE:
        _CACHE["nc"] = _build_bass()
    return _CACHE["nc"]


def _hilo16(x):
    """fp16 high/low split: x ~= hi + lo exactly to ~2^-22 relative."""
    x = np.asarray(x, np.float32)
    hi = x.astype(np.float16)
    lo = (x - hi.astype(np.float32)).astype(np.float16)
    return hi, lo


def _prep_inputs(feature_in, out, w1, b1, w2, b2):
    feature_in = np.asarray(feature_in, np.float32)
    out = np.asarray(out, np.float32)
    w1 = np.asarray(w1, np.float32)
    b1 = np.asarray(b1, np.float32)
    w2 = np.asarray(w2, np.float32)
    b2 = np.asarray(b2, np.float32)

    scale = np.float32(1.0 / np.sqrt(NCLS))
    feat = feature_in.reshape(NB, C_IN, HW)
    # f1 carries the softmax scale; f2 is plain
    f1 = (np.einsum("oc,ncp->nop", w1, feat, dtype=np.float32) + b1[None, :, None]) * scale
    f2 = np.einsum("oc,ncp->nop", w2, feat, dtype=np.float32) + b2[None, :, None]
    f1 = f1.astype(np.float32)
    f2 = f2.astype(np.float32)
    v = _resize_bilinear_ac(out, H, W).reshape(NB, NCLS, HW)

    in_maps = []
    for core in range(N_CORES):
        b, s = divmod(core, NSH)
        p0 = PSH * s
        f1p = np.zeros((32, PPAD), np.float16)
        h1, l1 = _hilo16(f1[b][:, p0 : p0 + PSH])
        f1p[0:4, :PSH] = h1
        f1p[4:8, :PSH] = l1
        f1p[8:12, :PSH] = h1
        h2, l2 = _hilo16(f2[b])
        f2p = np.zeros((32, HW), np.float16)
        f2p[0:12] = np.concatenate([h2, h2, l2], axis=0)
        vtp = np.zeros((NCLS, PPAD), np.float32)
        vtp[:, :PSH] = v[b][:, p0 : p0 + PSH]
        # vt[part, 4*pb + c] = V[c, p0 + 128*pb + part]
        vt = vtp.reshape(NCLS, PBLK, 128).transpose(2, 1, 0).reshape(128, PBLK * NCLS)
        in_maps.append(
            {
                "f1p": f1p,
                "f2p": np.ascontiguousarray(f2p),
                "vt": np.ascontiguousarray(vt),
                "zz": np.zeros((32, HW), np.float16),
            }
        )
    return in_maps


def _unpack(results):
    """results: list of 8 dicts with 'res' [52, 512] -> fina [2,4,80,80]."""
    fina = np.zeros((NB, NCLS, HW), np.float32)
    for core in range(N_CORES):
        b, _s = divmod(core, NSH)
        res = np.asarray(results[core]["res"], np.float32)  # [52, 512]
        for s in range(NSUB):
            w = min(512, HW - 512 * s)
            fina[b, :, 512 * s : 512 * s + w] += res[4 * s : 4 * s + 4, :w]
    fina *= np.float32(1.0 / 2048.0)
    return fina.reshape(NB, NCLS, H, W)


def run(inputs, trace=False):
    from concourse.bass_utils import run_bass_kernel_spmd

    nc = _get_nc()
    in_maps = _prep_inputs(**inputs)
    r = run_bass_kernel_spmd(nc, in_maps, list(range(N_CORES)), trace=trace)
    return _unpack(r.results), r.exec_time_ns


def kernel(feature_in, out, w1, b1, w2, b2):
    result, _ = run(
        dict(feature_in=feature_in, out=out, w1=w1, b1=b1, w2=w2, b2=b2)
    )
    return result


# revision 29
# speedup vs baseline: 1.3542x; 1.0105x over previous
"""Trainium2 Bass kernel for nn_Corr (correlation-attention module).

Math (per batch n):
    f1 = 0.5*(w1 @ feat + b1)        # [4, 6400]   feat = feature_in[n] flattened
    f2 =      w2 @ feat + b2         # [4, 6400]
    S  = f1^T @ f2                   # [6400, 6400]  (0.5 = 1/sqrt(nclass) folded into f1)
    A  = softmax(S, axis=1)          # row softmax (over q)
    V  = bilinear_resize(out[n])     # [4, 6400]
    fina[c, q] = sum_p V[c, p]/Z_p * exp(S[p, q])

Sharding: 2 batches x 4 p-shards (rows of S) = 8 cores. Each core produces a
partial fina over its 1600 p-rows; host sums the 4 partials per batch.

Device kernel per core, ScalarE(exp)-bound design (~6.6us per 128-row p-block):
  - S chunk = matmul(lhsT=f1pad[:, block], rhs=f2pad[:, qchunk]) with M=128,
    K=128 (12 live rows: fp16 hi/lo split), N=512 per MM (one PSUM bank).
  - Two ping-pong PSUM regions A=[128,2048] (4 banks) / B=[128,1536] (3 banks)
    give 4 wide EXP activations per block (2048/1536/2048/768), each with
    accum_out producing the row-sum partials Z for free.
  - fina accumulates IN PSUM (1 bank, [128,512]) across all 13 p-blocks:
    MM s (q-subgroup of 512) uses a zero-padded [128,128] lhsT slice of a
    sliding window buffer holding V/Z at cols 48:52, so output partitions
    4s..4s+4 catch class c of q-subgroup s and all other partitions get +0.
  - Emission order per iteration: S(pb) MMs+EXP, then fina(pb-1) MMs, then
    Z/vt scaling on VectorE -> PE never blocks the ACT stream.
"""

import numpy as np

N_CORES = 8
NB = 2          # batches
NCLS = 4        # nclass
C_IN = 32
H = W = 80
HW = H * W      # 6400
NSH = 4         # p-shards per batch
PSH = HW // NSH  # 1600 p rows per shard
PBLK = 13        # p blocks of 128 (1664 = 13*128, last 64 rows are zero-pad)
PPAD = PBLK * 128  # 1664
NSUB = 13        # fina q-subgroups of 512 (12*512 + 256)
# S chunk widths per block: ping-pong regions A (4 banks) / B (3 banks)
CHUNKS = ((0, 2048, "A"), (2048, 1536, "B"), (3584, 2048, "A"), (5632, 768, "B"))

_CACHE = {}


def _resize_bilinear_ac(x, h_out, w_out):
    """numpy mirror of the reference's align_corners=True bilinear resize."""
    n, c, h, w = x.shape
    if (h, w) == (h_out, w_out):
        return x
    ys = np.linspace(0.0, h - 1.0, h_out, dtype=np.float32)
    xs = np.linspace(0.0, w - 1.0, w_out, dtype=np.float32)
    y0 = np.floor(ys).astype(np.int32)
    x0 = np.floor(xs).astype(np.int32)
    y1 = np.minimum(y0 + 1, h - 1)
    x1 = np.minimum(x0 + 1, w - 1)
    wy = (ys - y0.astype(np.float32))[None, None, :, None]
    wx = (xs - x0.astype(np.float32))[None, None, None, :]
    g = lambda yi, xi: x[:, :, yi, :][:, :, :, xi]
    top = g(y0, x0) * (1.0 - wx) + g(y0, x1) * wx
    bot = g(y1, x0) * (1.0 - wx) + g(y1, x1) * wx
    return (top * (1.0 - wy) + bot * wy).astype(np.float32)


def _build_bass():
    import concourse.bass as bass
    import concourse.tile as tile
    from concourse import bacc, mybir

    f32 = mybir.dt.float32
    f16 = mybir.dt.float16

    nc = bacc.Bacc(
        "TRN2", target_bir_lowering=False, debug=False, num_devices=N_CORES
    )

    f1p_d = nc.dram_tensor("f1p", [32, PPAD], f16, kind="ExternalInput")
    f2p_d = nc.dram_tensor("f2p", [32, HW], f16, kind="ExternalInput")
    zz_d = nc.dram_tensor("zz", [32, HW], f16, kind="ExternalInput")
    vt_d = nc.dram_tensor("vt", [128, NCLS * PBLK], f32, kind="ExternalInput")
    res_d = nc.dram_tensor("res", [128, 512], f32, kind="ExternalOutput")

    EXP = mybir.ActivationFunctionType.Exp
    ADD = mybir.AluOpType.add
    MULT = mybir.AluOpType.mult
    AXX = mybir.AxisListType.X

    with tile.TileContext(nc) as tc:
        with (
            tc.tile_pool(name="const", bufs=1) as cpool,
            tc.tile_pool(name="estrip", bufs=2) as epool,
            tc.tile_pool(name="zpool", bufs=2) as zpool,
            tc.tile_pool(name="spsum", bufs=1, space="PSUM") as spool,
            tc.tile_pool(name="fpsum", bufs=1, space="PSUM") as fpool,
        ):
            f1s = cpool.tile([128, PPAD], f16, tag="f1s")
            f2s = cpool.tile([128, HW], f16, tag="f2s")
            vts = cpool.tile([128, NCLS * PBLK], f32, tag="vts")
            # sliding-window fina weights: V/Z at cols 48:52, zeros elsewhere;
            # MM s uses the [128,64] slice starting at col 48-4s
            vtbA = cpool.tile([128, 160], f16, tag="vtbA")
            vtbB = cpool.tile([128, 160], f16, tag="vtbB")
            bneg = cpool.tile([128, 1], f32, tag="bneg")
            fout = cpool.tile([128, 512], f32, tag="fout")
            dumm = cpool.tile([128, 1], f32, tag="dumm")

            # persistent fina accumulator: 1 PSUM bank
            fps = fpool.tile([128, 512], f32, tag="fps")

            # zeroing the K pad rows (12:128) is spread over DMA (host zeros),
            # GpSimd, and DVE so the head is limited only by the live-row DMAs
            u32 = mybir.dt.uint32
            nc.sync.dma_start(out=f1s[0:32, :], in_=f1p_d[:, :])
            nc.sync.dma_start(out=f2s[0:32, :], in_=f2p_d[:, :])
            nc.sync.dma_start(out=f2s[96:128, :], in_=zz_d[:, :])
            nc.sync.dma_start(out=vts[:, :], in_=vt_d[:, :])
            nc.gpsimd.memset(bneg[:, :], -5.0)
            # dummy activation: pulls the ~2.7us exp table load under the DMAs
            nc.scalar.activation(dumm[:, 0:1], bneg[:, 0:1], EXP)
            nc.sync.dma_start(out=f1s[96:128, :], in_=zz_d[:, 0:PPAD])
            nc.gpsimd.memset(vtbA[:, :], 0.0)
            nc.gpsimd.memset(vtbB[:, :], 0.0)
            nc.gpsimd.memset(f1s[32:64, :], 0.0)
            nc.gpsimd.memset(f1s[64:96, :], 0.0)
            nc.vector.memset(f2s[32:64, :].bitcast(u32), 0)
            nc.vector.memset(f2s[64:96, :].bitcast(u32), 0)
            # warm-up matmuls on zeroed tiles: PE activity until the input
            # DMAs land keeps the HAM clock gate at 8/8 for the real matmuls
            wst = spool.tile([128, 2048], f32, tag="stA")
            for _ in range(40):
                nc.tensor.matmul(
                    wst[0:64, 0:112],
                    lhsT=vtbB[:, 0:64],
                    rhs=vtbA[:, 0:112],
                    start=True,
                    stop=True,
                    skip_group_check=True,
                )
            # clear the whole fina bank once (start=True zero weights) so all
            # 169 fina accumulation matmuls can use start=False safely
            nc.tensor.matmul(
                fps[:, 0:512],
                lhsT=vtbA[:, 0:128],
                rhs=f2s[:, 0:512],
                start=True,
                stop=False,
                skip_group_check=True,
            )

            def emit_fina(pb):
                # subgroup s -> output partitions 64*(s%2) + 4*(s//2) + c:
                # consecutive MMs hit different PE column groups and overlap
                vtb = vtbA if pb % 2 == 0 else vtbB
                et = et_of[pb]
                for s in range(NSUB):
                    w = min(512, HW - 512 * s)
                    half = 64 * (s % 2)
                    k = s // 2
                    nc.tensor.matmul(
                        fps[half : half + 64, 0:w],
                        lhsT=vtb[:, 48 - 4 * k : 112 - 4 * k],
                        rhs=et[:, 512 * s : 512 * s + w],
                        start=False,
                        stop=(pb == PBLK - 1 and s == NSUB - 1),
                        skip_group_check=True,
                    )

            et_of = {}
            for pb in range(PBLK):
                et = epool.tile([128, HW], f16, tag="et")
                et_of[pb] = et
                zparts = zpool.tile([128, 4], f32, tag="zparts")
                rz = zpool.tile([128, 1], f32, tag="rz")

                for ci, (q0, width, reg) in enumerate(CHUNKS):
                    # fina(pb-1) emitted before the last S chunk: frees the
                    # PE to start S(pb+1) right after ACT(pb) region-A read
                    if ci == 3 and pb > 0:
                        emit_fina(pb - 1)
                    st = spool.tile([128, 2048 if reg == "A" else 1536], f32,
                                    tag="st" + reg)
                    for off in range(0, width, 512):
                        w = min(512, width - off)
                        # M=64 halves: 16-bit moving operand streams 2
                        # cols/cycle only when M <= 64 (PSUM drain port)
                        for h in range(2):
                            nc.tensor.matmul(
                                st[64 * h : 64 * h + 64, off : off + w],
                                lhsT=f1s[:, 128 * pb + 64 * h : 128 * pb + 64 * h + 64],
                                rhs=f2s[:, q0 + off : q0 + off + w],
                                start=True,
                                stop=True,
                                skip_group_check=True,
                            )
                    # bias -5: keeps exp within fp16 range (softmax is
                    # shift-invariant; Z accumulates the same shifted values)
                    nc.scalar.activation(
                        et[:, q0 : q0 + width],
                        st[:, 0:width],
                        EXP,
                        bias=bneg[:, 0:1],
                        accum_out=zparts[:, ci : ci + 1],
                    )

                # Z = sum of chunk partials; vtb cols 48:52 = V[:, block]/Z
                nc.vector.tensor_reduce(rz[:, 0:1], zparts[:, :], AXX, ADD)
                nc.vector.reciprocal(rz[:, 0:1], rz[:, 0:1])
                vtb = vtbA if pb % 2 == 0 else vtbB
                nc.vector.tensor_scalar(
                    vtb[:, 48:52],
                    vts[:, NCLS * pb : NCLS * pb + NCLS],
                    rz[:, 0:1],
                    2048.0,
                    MULT,
                    MULT,
                )

            # keep the PE warm through the last ACT window (no S(pb+1) work
            # exists) so fina(12) runs at full clock instead of HAM-throttled
            wtl = spool.tile([128, 2048], f32, tag="stA")
            for _ in range(10):
                nc.tensor.matmul(
                    wtl[0:64, 0:512],
                    lhsT=vtbB[:, 0:64],
                    rhs=f2s[:, 0:512],
                    start=True,
                    stop=True,
                    skip_group_check=True,
                )
            emit_fina(PBLK - 1)
            nc.vector.tensor_copy(fout[:, :], fps[:, :])
            nc.sync.dma_start(out=res_d[:, :], in_=fout[:, :])

    nc.compile()
    return nc


def _get_nc():
    if "nc" not in _CACHE:
        _CACHE["nc"] = _build_bass()
    return _CACHE["nc"]


def _hilo16(x):
    """fp16 high/low split: x ~= hi + lo exactly to ~2^-22 relative."""
    x = np.asarray(x, np.float32)
    hi = x.astype(np.float16)
    lo = (x - hi.astype(np.float32)).astype(np.float16)
    return hi, lo


def _prep_inputs(feature_in, out, w1, b1, w2, b2):
    feature_in = np.asarray(feature_in, np.float32)
    out = np.asarray(out, np.float32)
    w1 = np.asarray(w1, np.float32)
    b1 = np.asarray(b1, np.float32)
    w2 = np.asarray(w2, np.float32)
    b2 = np.asarray(b2, np.float32)

    scale = np.float32(1.0 / np.sqrt(NCLS))
    feat = feature_in.reshape(NB, C_IN, HW)
    # f1 carries the softmax scale; f2 is plain
    f1 = (np.einsum("oc,ncp->nop", w1, feat, dtype=np.float32) + b1[None, :, None]) * scale
    f2 = np.einsum("oc,ncp->nop", w2, feat, dtype=np.float32) + b2[None, :, None]
    f1 = f1.astype(np.float32)
    f2 = f2.astype(np.float32)
    v = _resize_bilinear_ac(out, H, W).reshape(NB, NCLS, HW)

    in_maps = []
    for core in range(N_CORES):
        b, s = divmod(core, NSH)
        p0 = PSH * s
        f1p = np.zeros((32, PPAD), np.float16)
        h1, l1 = _hilo16(f1[b][:, p0 : p0 + PSH])
        f1p[0:4, :PSH] = h1
        f1p[4:8, :PSH] = l1
        f1p[8:12, :PSH] = h1
        h2, l2 = _hilo16(f2[b])
        f2p = np.zeros((32, HW), np.float16)
        f2p[0:12] = np.concatenate([h2, h2, l2], axis=0)
        vtp = np.zeros((NCLS, PPAD), np.float32)
        vtp[:, :PSH] = v[b][:, p0 : p0 + PSH]
        # vt[part, 4*pb + c] = V[c, p0 + 128*pb + part]
        vt = vtp.reshape(NCLS, PBLK, 128).transpose(2, 1, 0).reshape(128, PBLK * NCLS)
        in_maps.append(
            {
                "f1p": f1p,
                "f2p": np.ascontiguousarray(f2p),
                "vt": np.ascontiguousarray(vt),
                "zz": np.zeros((32, HW), np.float16),
            }
        )
    return in_maps


def _unpack(results):
    """results: list of 8 dicts with 'res' [128, 512] -> fina [2,4,80,80]."""
    fina = np.zeros((NB, NCLS, HW), np.float32)
    for core in range(N_CORES):
        b, _s = divmod(core, NSH)
        res = np.asarray(results[core]["res"], np.float32)  # [128, 512]
        for s in range(NSUB):
            w = min(512, HW - 512 * s)
            r0 = 64 * (s % 2) + 4 * (s // 2)
            fina[b, :, 512 * s : 512 * s + w] += res[r0 : r0 + 4, :w]
    fina *= np.float32(1.0 / 2048.0)
    return fina.reshape(NB, NCLS, H, W)


def run(inputs, trace=False):
    from concourse.bass_utils import run_bass_kernel_spmd

    nc = _get_nc()
    in_maps = _prep_inputs(**inputs)
    r = run_bass_kernel_spmd(nc, in_maps, list(range(N_CORES)), trace=trace)
    return _unpack(r.results), r.exec_time_ns


def kernel(feature_in, out, w1, b1, w2, b2):
    result, _ = run(
        dict(feature_in=feature_in, out=out, w1=w1, b1=b1, w2=w2, b2=b2)
    )
    return result


# revision 31
# speedup vs baseline: 1.4127x; 1.0432x over previous
"""Trainium2 Bass kernel for nn_Corr (correlation-attention module).

Math (per batch n):
    f1 = 0.5*(w1 @ feat + b1)        # [4, 6400]   feat = feature_in[n] flattened
    f2 =      w2 @ feat + b2         # [4, 6400]
    S  = f1^T @ f2                   # [6400, 6400]  (0.5 = 1/sqrt(nclass) folded into f1)
    A  = softmax(S, axis=1)          # row softmax (over q)
    V  = bilinear_resize(out[n])     # [4, 6400]
    fina[c, q] = sum_p V[c, p]/Z_p * exp(S[p, q])

Sharding: 2 batches x 4 p-shards (rows of S) = 8 cores. Each core produces a
partial fina over its 1600 p-rows; host sums the 4 partials per batch.

Device kernel per core, ScalarE(exp)-bound design (~6.6us per 128-row p-block):
  - S chunk = matmul(lhsT=f1pad[:, block], rhs=f2pad[:, qchunk]) with M=128,
    K=128 (12 live rows: fp16 hi/lo split), N=512 per MM (one PSUM bank).
  - Two ping-pong PSUM regions A=[128,2048] (4 banks) / B=[128,1536] (3 banks)
    give 4 wide EXP activations per block (2048/1536/2048/768), each with
    accum_out producing the row-sum partials Z for free.
  - fina accumulates IN PSUM (1 bank, [128,512]) across all 13 p-blocks:
    MM s (q-subgroup of 512) uses a zero-padded [128,128] lhsT slice of a
    sliding window buffer holding V/Z at cols 48:52, so output partitions
    4s..4s+4 catch class c of q-subgroup s and all other partitions get +0.
  - Emission order per iteration: S(pb) MMs+EXP, then fina(pb-1) MMs, then
    Z/vt scaling on VectorE -> PE never blocks the ACT stream.
"""

import numpy as np

N_CORES = 8
NB = 2          # batches
NCLS = 4        # nclass
C_IN = 32
H = W = 80
HW = H * W      # 6400
NSH = 4         # p-shards per batch
PSH = HW // NSH  # 1600 p rows per shard
PBLK = 13        # p blocks of 128 (1664 = 13*128, last 64 rows are zero-pad)
PPAD = PBLK * 128  # 1664
NSUB = 13        # fina q-subgroups of 512 (12*512 + 256)
# S chunk widths per block: ping-pong regions A (4 banks) / B (3 banks);
# near-uniform widths keep every S-fill inside the other region's ACT window
CHUNKS = ((0, 1664, "A"), (1664, 1536, "B"), (3200, 1664, "A"), (4864, 1536, "B"))

_CACHE = {}


def _resize_bilinear_ac(x, h_out, w_out):
    """numpy mirror of the reference's align_corners=True bilinear resize."""
    n, c, h, w = x.shape
    if (h, w) == (h_out, w_out):
        return x
    ys = np.linspace(0.0, h - 1.0, h_out, dtype=np.float32)
    xs = np.linspace(0.0, w - 1.0, w_out, dtype=np.float32)
    y0 = np.floor(ys).astype(np.int32)
    x0 = np.floor(xs).astype(np.int32)
    y1 = np.minimum(y0 + 1, h - 1)
    x1 = np.minimum(x0 + 1, w - 1)
    wy = (ys - y0.astype(np.float32))[None, None, :, None]
    wx = (xs - x0.astype(np.float32))[None, None, None, :]
    g = lambda yi, xi: x[:, :, yi, :][:, :, :, xi]
    top = g(y0, x0) * (1.0 - wx) + g(y0, x1) * wx
    bot = g(y1, x0) * (1.0 - wx) + g(y1, x1) * wx
    return (top * (1.0 - wy) + bot * wy).astype(np.float32)


def _build_bass():
    import concourse.bass as bass
    import concourse.tile as tile
    from concourse import bacc, mybir

    f32 = mybir.dt.float32
    f16 = mybir.dt.float16

    nc = bacc.Bacc(
        "TRN2", target_bir_lowering=False, debug=False, num_devices=N_CORES
    )

    f1p_d = nc.dram_tensor("f1p", [32, PPAD], f16, kind="ExternalInput")
    f2p_d = nc.dram_tensor("f2p", [32, HW], f16, kind="ExternalInput")
    zz_d = nc.dram_tensor("zz", [32, HW], f16, kind="ExternalInput")
    vt_d = nc.dram_tensor("vt", [128, NCLS * PBLK], f32, kind="ExternalInput")
    res_d = nc.dram_tensor("res", [128, 512], f32, kind="ExternalOutput")

    EXP = mybir.ActivationFunctionType.Exp
    ADD = mybir.AluOpType.add
    MULT = mybir.AluOpType.mult
    AXX = mybir.AxisListType.X

    with tile.TileContext(nc) as tc:
        with (
            tc.tile_pool(name="const", bufs=1) as cpool,
            tc.tile_pool(name="estrip", bufs=2) as epool,
            tc.tile_pool(name="zpool", bufs=2) as zpool,
            tc.tile_pool(name="spsum", bufs=1, space="PSUM") as spool,
            tc.tile_pool(name="fpsum", bufs=1, space="PSUM") as fpool,
        ):
            f1s = cpool.tile([128, PPAD], f16, tag="f1s")
            f2s = cpool.tile([128, HW], f16, tag="f2s")
            vts = cpool.tile([128, NCLS * PBLK], f32, tag="vts")
            # sliding-window fina weights: V/Z at cols 48:52, zeros elsewhere;
            # MM s uses the [128,64] slice starting at col 48-4s
            vtbA = cpool.tile([128, 160], f16, tag="vtbA")
            vtbB = cpool.tile([128, 160], f16, tag="vtbB")
            bneg = cpool.tile([128, 1], f32, tag="bneg")
            fout = cpool.tile([128, 512], f32, tag="fout")
            dumm = cpool.tile([128, 1], f32, tag="dumm")

            # persistent fina accumulator: 1 PSUM bank
            fps = fpool.tile([128, 512], f32, tag="fps")

            # zeroing the K pad rows (12:128) is spread over DMA (host zeros),
            # GpSimd, and DVE so the head is limited only by the live-row DMAs
            u32 = mybir.dt.uint32
            nc.sync.dma_start(out=f1s[0:32, :], in_=f1p_d[:, :])
            nc.sync.dma_start(out=f2s[0:32, :], in_=f2p_d[:, :])
            nc.sync.dma_start(out=f2s[96:128, :], in_=zz_d[:, :])
            nc.sync.dma_start(out=vts[:, :], in_=vt_d[:, :])
            nc.gpsimd.memset(bneg[:, :], -5.0)
            # dummy activation: pulls the ~2.7us exp table load under the DMAs
            nc.scalar.activation(dumm[:, 0:1], bneg[:, 0:1], EXP)
            nc.sync.dma_start(out=f1s[96:128, :], in_=zz_d[:, 0:PPAD])
            nc.gpsimd.memset(vtbA[:, :], 0.0)
            nc.gpsimd.memset(vtbB[:, :], 0.0)
            nc.gpsimd.memset(f1s[32:64, :], 0.0)
            nc.gpsimd.memset(f1s[64:96, :], 0.0)
            nc.vector.memset(f2s[32:64, :].bitcast(u32), 0)
            nc.scalar.memzero(f2s[64:96, :])
            # warm-up matmuls on zeroed tiles: PE activity until the input
            # DMAs land keeps the HAM clock gate at 8/8 for the real matmuls
            wst = spool.tile([128, 2048], f32, tag="stA")
            for _ in range(40):
                nc.tensor.matmul(
                    wst[0:64, 0:112],
                    lhsT=vtbB[:, 0:64],
                    rhs=vtbA[:, 0:112],
                    start=True,
                    stop=True,
                    skip_group_check=True,
                )
            # clear the whole fina bank once (start=True zero weights) so all
            # 169 fina accumulation matmuls can use start=False safely
            nc.tensor.matmul(
                fps[:, 0:512],
                lhsT=vtbA[:, 0:128],
                rhs=f2s[:, 0:512],
                start=True,
                stop=False,
                skip_group_check=True,
            )

            def emit_fina(pb):
                # subgroup s -> output partitions 64*(s%2) + 4*(s//2) + c:
                # consecutive MMs hit different PE column groups and overlap
                vtb = vtbA if pb % 2 == 0 else vtbB
                et = et_of[pb]
                for s in range(NSUB):
                    w = min(512, HW - 512 * s)
                    half = 64 * (s % 2)
                    k = s // 2
                    nc.tensor.matmul(
                        fps[half : half + 64, 0:w],
                        lhsT=vtb[:, 48 - 4 * k : 112 - 4 * k],
                        rhs=et[:, 512 * s : 512 * s + w],
                        start=False,
                        stop=(pb == PBLK - 1 and s == NSUB - 1),
                        skip_group_check=True,
                    )

            et_of = {}
            for pb in range(PBLK):
                et = epool.tile([128, HW], f16, tag="et")
                et_of[pb] = et
                zparts = zpool.tile([128, 4], f32, tag="zparts")
                rz = zpool.tile([128, 1], f32, tag="rz")

                for ci, (q0, width, reg) in enumerate(CHUNKS):
                    # fina(pb-1) emitted before the last S chunk: frees the
                    # PE to start S(pb+1) right after ACT(pb) region-A read
                    if ci == 3 and pb > 0:
                        emit_fina(pb - 1)
                    st = spool.tile([128, 2048 if reg == "A" else 1536], f32,
                                    tag="st" + reg)
                    for off in range(0, width, 512):
                        w = min(512, width - off)
                        # M=64 halves: 16-bit moving operand streams 2
                        # cols/cycle only when M <= 64 (PSUM drain port)
                        for h in range(2):
                            nc.tensor.matmul(
                                st[64 * h : 64 * h + 64, off : off + w],
                                lhsT=f1s[:, 128 * pb + 64 * h : 128 * pb + 64 * h + 64],
                                rhs=f2s[:, q0 + off : q0 + off + w],
                                start=True,
                                stop=True,
                                skip_group_check=True,
                            )
                    # bias -5: keeps exp within fp16 range (softmax is
                    # shift-invariant; Z accumulates the same shifted values)
                    nc.scalar.activation(
                        et[:, q0 : q0 + width],
                        st[:, 0:width],
                        EXP,
                        bias=bneg[:, 0:1],
                        accum_out=zparts[:, ci : ci + 1],
                    )

                # Z = sum of chunk partials; vtb cols 48:52 = V[:, block]/Z
                nc.vector.tensor_reduce(rz[:, 0:1], zparts[:, :], AXX, ADD)
                nc.vector.reciprocal(rz[:, 0:1], rz[:, 0:1])
                vtb = vtbA if pb % 2 == 0 else vtbB
                nc.vector.tensor_scalar(
                    vtb[:, 48:52],
                    vts[:, NCLS * pb : NCLS * pb + NCLS],
                    rz[:, 0:1],
                    2048.0,
                    MULT,
                    MULT,
                )

            # keep the PE warm through the last ACT window (no S(pb+1) work
            # exists) so fina(12) runs at full clock instead of HAM-throttled
            wtl = spool.tile([128, 2048], f32, tag="stA")
            for _ in range(10):
                nc.tensor.matmul(
                    wtl[0:64, 0:512],
                    lhsT=vtbB[:, 0:64],
                    rhs=f2s[:, 0:512],
                    start=True,
                    stop=True,
                    skip_group_check=True,
                )
            emit_fina(PBLK - 1)
            nc.vector.tensor_copy(fout[:, :], fps[:, :])
            nc.sync.dma_start(out=res_d[:, :], in_=fout[:, :])

    nc.compile()
    return nc


def _get_nc():
    if "nc" not in _CACHE:
        _CACHE["nc"] = _build_bass()
    return _CACHE["nc"]


def _hilo16(x):
    """fp16 high/low split: x ~= hi + lo exactly to ~2^-22 relative."""
    x = np.asarray(x, np.float32)
    hi = x.astype(np.float16)
    lo = (x - hi.astype(np.float32)).astype(np.float16)
    return hi, lo


def _prep_inputs(feature_in, out, w1, b1, w2, b2):
    feature_in = np.asarray(feature_in, np.float32)
    out = np.asarray(out, np.float32)
    w1 = np.asarray(w1, np.float32)
    b1 = np.asarray(b1, np.float32)
    w2 = np.asarray(w2, np.float32)
    b2 = np.asarray(b2, np.float32)

    scale = np.float32(1.0 / np.sqrt(NCLS))
    feat = feature_in.reshape(NB, C_IN, HW)
    # f1 carries the softmax scale; f2 is plain
    f1 = (np.einsum("oc,ncp->nop", w1, feat, dtype=np.float32) + b1[None, :, None]) * scale
    f2 = np.einsum("oc,ncp->nop", w2, feat, dtype=np.float32) + b2[None, :, None]
    f1 = f1.astype(np.float32)
    f2 = f2.astype(np.float32)
    v = _resize_bilinear_ac(out, H, W).reshape(NB, NCLS, HW)

    in_maps = []
    for core in range(N_CORES):
        b, s = divmod(core, NSH)
        p0 = PSH * s
        f1p = np.zeros((32, PPAD), np.float16)
        h1, l1 = _hilo16(f1[b][:, p0 : p0 + PSH])
        f1p[0:4, :PSH] = h1
        f1p[4:8, :PSH] = l1
        f1p[8:12, :PSH] = h1
        h2, l2 = _hilo16(f2[b])
        f2p = np.zeros((32, HW), np.float16)
        f2p[0:12] = np.concatenate([h2, h2, l2], axis=0)
        vtp = np.zeros((NCLS, PPAD), np.float32)
        vtp[:, :PSH] = v[b][:, p0 : p0 + PSH]
        # vt[part, 4*pb + c] = V[c, p0 + 128*pb + part]
        vt = vtp.reshape(NCLS, PBLK, 128).transpose(2, 1, 0).reshape(128, PBLK * NCLS)
        in_maps.append(
            {
                "f1p": f1p,
                "f2p": np.ascontiguousarray(f2p),
                "vt": np.ascontiguousarray(vt),
                "zz": np.zeros((32, HW), np.float16),
            }
        )
    return in_maps


def _unpack(results):
    """results: list of 8 dicts with 'res' [128, 512] -> fina [2,4,80,80]."""
    fina = np.zeros((NB, NCLS, HW), np.float32)
    for core in range(N_CORES):
        b, _s = divmod(core, NSH)
        res = np.asarray(results[core]["res"], np.float32)  # [128, 512]
        for s in range(NSUB):
            w = min(512, HW - 512 * s)
            r0 = 64 * (s % 2) + 4 * (s // 2)
            fina[b, :, 512 * s : 512 * s + w] += res[r0 : r0 + 4, :w]
    fina *= np.float32(1.0 / 2048.0)
    return fina.reshape(NB, NCLS, H, W)


def run(inputs, trace=False):
    from concourse.bass_utils import run_bass_kernel_spmd

    nc = _get_nc()
    in_maps = _prep_inputs(**inputs)
    r = run_bass_kernel_spmd(nc, in_maps, list(range(N_CORES)), trace=trace)
    return _unpack(r.results), r.exec_time_ns


def kernel(feature_in, out, w1, b1, w2, b2):
    result, _ = run(
        dict(feature_in=feature_in, out=out, w1=w1, b1=b1, w2=w2, b2=b2)
    )
    return result
